# revision 1
# baseline (speedup 1.0000x reference)
"""Dense transformer block (RMSNorm+MHA+residual, RMSNorm+SwiGLU+residual)
on 8 trn2 NeuronCores. Sharding: 2 cores per batch element; each core
computes the block output for 1024 of its batch's 2048 tokens, redundantly
computing K/V for the full sequence (attention keys are permutation
invariant, so each core's xT puts its own 1024 query tokens first).
No inter-core communication.

All on-chip tensors are feature-major ([feature, token]) so every matmul
contraction lands on the partition dim. Softmax denominators come free
from a ones-column appended to V.

Optimizations over the original baseline (972us -> 800us cost-model
makespan, accuracy preserved at ~9e-4 max abs err):
- Deferred RMSNorm: Q/K/V projections run on raw x (the per-token norm
  scale commutes with the d-contraction) and the scale is applied at
  PSUM evacuation -- DVE multiply by a broadcast row for K/Q,
  scalar-engine Copy with per-partition scale for token-major V.
  Projections start ~7us after the first x slice lands.
- Softmax exp batched over [128, 2x512] PSUM groups (amortizes the
  ~293-cycle per-ACTIVATE overhead) with double-buffered score tiles so
  scores/exp/attnV pipeline; the scalar engine runs at ~0.99 occupancy
  through the attention window.
- silu via tanh identity (0.5*g*(1+tanh(g/2))) keeps the scalar engine
  on one activation table set; rsqrt via Ln+Exp avoids the Sqrt table.
- GpSimd partition_broadcast replaces PSUM broadcast matmuls.
- FFN (f22/f32r weights for accuracy) is quarter-of-hidden structured,
  accumulates the output into x1T in place, and runs with fully
  double-buffered PSUM (alternating gate/hidden pools + dedicated
  out-projection pool): tensor engine at 1.00 through the FFN phase.
- The K/Q/V SBUF pool closes after attention so the FFN pools reuse its
  81KB; all weight/x DMAs are single strided transfers spread across
  engine queues.
PSUM budget (8 banks) in the attention window: scores 2x2 + attnV acc 1
+ Wo/ss 1 + gate/hidden prefill 1+1 -- every rearrangement of this
allocation measured slower.
"""
import sys
from contextlib import ExitStack

import numpy as np

sys.path.insert(0, "/opt/trn_rl_repo")

import ml_dtypes  # noqa: E402
import concourse.bass as bass  # noqa: E402
from concourse import bacc  # noqa: E402
import concourse.tile as tile  # noqa: E402
from concourse import mybir  # noqa: E402
from concourse import bass_utils  # noqa: E402

P = 128
D = 1024          # d_model
L = 2048          # full seq per core (keys)
LQ = 1024         # query tokens per core
NH = 16
HD = 64
HID = 4096
EPS = 1e-6
NDT = D // P      # 8 feature tiles
NKT = L // P      # 16 key tiles
NHT = HID // P    # 32 hidden tiles
G = 2             # kt blocks per exp group (2 PSUM banks)
GROUPS = [(k, 2) for k in range(0, 16, 2)]
SW = 64.0         # fp8 weight scale (wg8/wh8/wo8 = 64*w)
SGH = 16.0        # ghT8 = 16 * silu(g) * h
F32 = mybir.dt.float32
BF16 = mybir.dt.bfloat16
FP8 = mybir.dt.float8e4
F32R = mybir.dt.float32r
AF = mybir.ActivationFunctionType
ALU = mybir.AluOpType
DR = mybir.MatmulPerfMode.DoubleRow

SIM_TIME_NS = None


def build_nc():
    global SIM_TIME_NS
    nc = bacc.Bacc(None, target_bir_lowering=False)
    d = {}
    d["xT"] = nc.dram_tensor("xT", [D, L], F32, kind="ExternalInput")
    d["wqT"] = nc.dram_tensor("wqT", [D, D], BF16, kind="ExternalInput")
    d["wkT"] = nc.dram_tensor("wkT", [D, D], BF16, kind="ExternalInput")
    d["wvT"] = nc.dram_tensor("wvT", [D, D], BF16, kind="ExternalInput")
    d["woT"] = nc.dram_tensor("woT", [D, D], BF16, kind="ExternalInput")
    d["bo_t"] = nc.dram_tensor("bo_t", [P, NDT], F32, kind="ExternalInput")
    d["wgT"] = nc.dram_tensor("wgT", [D, HID], F32R, kind="ExternalInput")
    d["whT"] = nc.dram_tensor("whT", [D, HID], F32R, kind="ExternalInput")
    d["woB"] = nc.dram_tensor("woB", [HID, D], F32R, kind="ExternalInput")
    d["bout_t"] = nc.dram_tensor("bout_t", [P, NDT], F32, kind="ExternalInput")
    d["yT"] = nc.dram_tensor("yT", [D, LQ], F32, kind="ExternalOutput")

    with tile.TileContext(nc) as tc:
        _body(tc, nc, d)
        _, snap = tc.schedule_and_allocate()
        SIM_TIME_NS = snap.time
    nc.compile()
    return nc


def _body(tc, nc, d):
    xTr = d["xT"].rearrange("(dt p) l -> p dt l", p=P)
    with ExitStack() as pp_ctx:
        pp = pp_ctx.enter_context(tc.tile_pool(name="persist", bufs=1))
        bo_sb = pp.tile([P, NDT], F32, tag="bo")
        bout_sb = pp.tile([P, NDT], F32, tag="bout")
        ones_col = pp.tile([P, 1], BF16, tag="ones")
        eps_t = pp.tile([1, 1], F32, tag="eps")
        x1T = pp.tile([P, NDT, LQ], F32, tag="x1T")
        x1np_o = pp_ctx.enter_context(tc.tile_pool(name="x1n", bufs=2))
        ghq0p = pp_ctx.enter_context(tc.tile_pool(name="ghq0", bufs=1))
        # gate/hidden/misc PSUM pools span attention (Wo/ss2) and FFN
        gpsp = pp_ctx.enter_context(tc.tile_pool(name="gps", bufs=1, space="PSUM"))
        hpsp = pp_ctx.enter_context(tc.tile_pool(name="hps", bufs=1, space="PSUM"))
        mpsp = pp_ctx.enter_context(tc.tile_pool(name="mps", bufs=1, space="PSUM"))
        nc.sync.dma_start(out=bo_sb, in_=d["bo_t"][:, :])
        nc.sync.dma_start(out=bout_sb, in_=d["bout_t"][:, :])
        nc.vector.memset(ones_col, 1.0)
        nc.vector.memset(eps_t, EPS)
        x1ns = []

        # kqv pool closes after attention so FFN pools reuse its SBUF
        with tc.tile_pool(name="kqv", bufs=1) as kqvp:
            kT = kqvp.tile([P, NDT, L], BF16, tag="kT")
            qT = kqvp.tile([P, NDT, LQ], BF16, tag="qT")
            vt = kqvp.tile([P, NKT, NH, HD + 1], BF16, tag="vt")
            nc.vector.memset(vt[:, :, :, HD:HD + 1], 1.0)

            # ---- phase 0/1: load x, rmsnorm in place, project K/Q/V ----
            with tc.tile_pool(name="xt", bufs=1) as xtp, \
                 tc.tile_pool(name="bc1", bufs=1) as bc1p, \
                 tc.tile_pool(name="rsc", bufs=1, space="DRAM") as rscp:
                xt = xtp.tile([P, NDT, L], BF16, tag="xt")
                bc = bc1p.tile([P, L], F32, tag="bc1")
                for ls in range(L // 512):
                    sl = slice(ls * 512, (ls + 1) * 512)
                    nc.gpsimd.dma_start(out=xt[:, :, sl], in_=xTr[:, :, sl])
                rsc = rscp.tile([L], F32, tag="rsc")
                with tc.tile_pool(name="n1", bufs=3) as n1p, \
                     tc.tile_pool(name="ss1", bufs=1, space="PSUM") as ssp:
                    ss_ps = ssp.tile([1, L], F32, tag="ss")
                    for ls in range(L // 512):
                        sl = slice(ls * 512, (ls + 1) * 512)
                        for dt_ in range(NDT):
                            sq = n1p.tile([P, 512], BF16, tag="sq")
                            nc.vector.tensor_mul(sq, xt[:, dt_, sl],
                                                 xt[:, dt_, sl])
                            nc.tensor.matmul(ss_ps[:, sl], ones_col, sq,
                                             start=(dt_ == 0),
                                             stop=(dt_ == NDT - 1))
                        lnr = bc1p.tile([1, 512], F32, tag="lnr")
                        nc.scalar.activation(lnr, ss_ps[:, sl], AF.Ln,
                                             bias=eps_t, scale=1.0 / D)
                        rr = bc1p.tile([1, 512], F32, tag="rr1")
                        nc.scalar.activation(rr, lnr, AF.Exp, scale=-0.5)
                        nc.gpsimd.partition_broadcast(bc[:, sl], rr)
                        nc.sync.dma_start(out=rsc[sl], in_=rr)
                rcol = bc1p.tile([P, NKT], F32, tag="rcol")
                nc.sync.dma_start(
                    out=rcol, in_=rsc.rearrange("(tt p) -> p tt", p=P))

                # K, Q projections (K first: scores depend on it)
                with tc.tile_pool(name="wblk", bufs=2) as wp, \
                     tc.tile_pool(name="wv", bufs=1) as wvp, \
                     tc.tile_pool(name="proj", bufs=4, space="PSUM") as prp:
                    for (w_d, dst, ntok) in ((d["wkT"], kT, L),
                                             (d["wqT"], qT, LQ)):
                        wr = w_d.rearrange("(dt p) f -> p dt f", p=P)
                        for ft in range(NDT):
                            wblk = wp.tile([P, NDT, P], BF16, tag="wblk")
                            nc.sync.dma_start(
                                out=wblk, in_=wr[:, :, ft * P:(ft + 1) * P])
                            for ns in range(ntok // 512):
                                sl = slice(ns * 512, (ns + 1) * 512)
                                ps = prp.tile([P, 512], F32, tag="pp")
                                for dt_ in range(NDT):
                                    nc.tensor.matmul(
                                        ps, wblk[:, dt_, :], xt[:, dt_, sl],
                                        start=(dt_ == 0),
                                        stop=(dt_ == NDT - 1))
                                nc.vector.tensor_mul(dst[:, ft, sl], ps,
                                                      bc[:, sl])
                    wvr = d["wvT"].rearrange("(dt p) f -> p dt f", p=P)
                    for hf in range(2):
                        wv = wvp.tile([P, NDT, 512], BF16, tag="wv")
                        nc.sync.dma_start(
                            out=wv, in_=wvr[:, :, hf * 512:(hf + 1) * 512])
                        for tt in range(NKT):
                            ps = prp.tile([P, 512], F32, tag="pp")
                            for dt_ in range(NDT):
                                nc.tensor.matmul(
                                    ps, xt[:, dt_, tt * P:(tt + 1) * P],
                                    wv[:, dt_, :],
                                    start=(dt_ == 0), stop=(dt_ == NDT - 1))
                            nc.scalar.activation(
                                vt[:, tt, hf * 8:(hf + 1) * 8, 0:HD],
                                ps.rearrange("p (h e) -> p h e", h=8),
                                AF.Copy, scale=rcol[:, tt:tt + 1])

            # ---- phase 2: attention + Wo + norm2 per 512-token slice ----
            with ExitStack() as ectx:
                atp = ectx.enter_context(tc.tile_pool(name="attnp", bufs=1))
                ptp = ectx.enter_context(tc.tile_pool(name="pt", bufs=3))
                smp = ectx.enter_context(tc.tile_pool(name="sm", bufs=2))
                rbp = ectx.enter_context(tc.tile_pool(name="rb", bufs=1))
                xqp = ectx.enter_context(tc.tile_pool(name="xq", bufs=3))
                sqp = ectx.enter_context(tc.tile_pool(name="sq2p", bufs=2))
                wop = ectx.enter_context(tc.tile_pool(name="wo", bufs=3))
                stp = ectx.enter_context(
                    tc.tile_pool(name="st", bufs=2, space="PSUM"))
                accp = ectx.enter_context(
                    tc.tile_pool(name="acc", bufs=1, space="PSUM"))
                for ns in range(2):
                    qsl = slice(ns * 512, (ns + 1) * 512)
                    attnT = atp.tile([P, NDT, 512], BF16, tag="attnT")
                    for h in range(NH):
                        dt_ = h // 2
                        r0 = (h % 2) * HD
                        acc = accp.tile([HD + 1, 512], F32, tag="acc")
                        for (k0, glen) in GROUPS:
                            st = stp.tile([P, glen, 512], F32, tag="st")
                            for j in range(glen):
                                kt = k0 + j
                                nc.tensor.matmul(
                                    st[:, j, :],
                                    kT[r0:r0 + HD, dt_, kt * P:(kt + 1) * P],
                                    qT[r0:r0 + HD, dt_, qsl],
                                    start=True, stop=True)
                            pt = ptp.tile([P, glen, 512], BF16, tag="pt")
                            nc.scalar.activation(pt, st, AF.Exp)
                            for j in range(glen):
                                kt = k0 + j
                                nc.tensor.matmul(
                                    acc, vt[:, kt, h, :], pt[:, j, :],
                                    start=(kt == 0), stop=(kt == NKT - 1))
                        accS = smp.tile([HD + 1, 512], F32, tag="accS")
                        nc.vector.tensor_copy(accS, acc)
                        rrow = smp.tile([1, 512], F32, tag="row")
                        nc.vector.reciprocal(rrow, accS[HD:HD + 1, :])
                        rb = rbp.tile([HD, 512], F32, tag="rb")
                        nc.gpsimd.partition_broadcast(rb, rrow)
                        nc.vector.tensor_mul(
                            attnT[r0:r0 + HD, dt_, :], accS[0:HD, :], rb)

                    # Wo projection + residual -> x1T
                    wor = d["woT"].rearrange("(dt p) f -> p dt f", p=P)
                    for ft in range(NDT):
                        wblk = wop.tile([P, NDT, P], BF16, tag="woblk")
                        nc.sync.dma_start(
                            out=wblk, in_=wor[:, :, ft * P:(ft + 1) * P])
                        xq = xqp.tile([P, 512], F32, tag="xq")
                        nc.gpsimd.dma_start(out=xq, in_=xTr[:, ft, qsl])
                        ps = mpsp.tile([P, 512], F32, tag="m")
                        for fi in range(NDT):
                            nc.tensor.matmul(
                                ps, wblk[:, fi, :], attnT[:, fi, :],
                                start=(fi == 0), stop=(fi == NDT - 1))
                        nc.vector.scalar_tensor_tensor(
                            out=x1T[:, ft, qsl], in0=ps,
                            scalar=bo_sb[:, ft:ft + 1],
                            in1=xq, op0=ALU.add, op1=ALU.add)

                    # rmsnorm2 -> x1n (f32r for the f22 FFN)
                    ss2 = mpsp.tile([1, 512], F32, tag="m")
                    for dt_ in range(NDT):
                        sq2 = sqp.tile([P, 512], BF16, tag="sq2")
                        nc.vector.tensor_mul(
                            sq2, x1T[:, dt_, qsl], x1T[:, dt_, qsl])
                        nc.tensor.matmul(ss2, ones_col, sq2,
                                         start=(dt_ == 0),
                                         stop=(dt_ == NDT - 1))
                    ln2 = smp.tile([1, 512], F32, tag="row")
                    nc.scalar.activation(ln2, ss2, AF.Ln, bias=eps_t,
                                         scale=1.0 / D)
                    rr2 = smp.tile([1, 512], F32, tag="row")
                    nc.scalar.activation(rr2, ln2, AF.Exp, scale=-0.5)
                    bc2 = rbp.tile([P, 512], F32, tag="rb")
                    nc.gpsimd.partition_broadcast(bc2, rr2)
                    x1n = x1np_o.tile([P, NDT, 512], F32R, tag="x1n")
                    for dt_ in range(NDT):
                        nc.vector.tensor_mul(
                            x1n[:, dt_, :], x1T[:, dt_, qsl], bc2)
                    x1ns.append(x1n)
                    if ns == 1:
                        # FFN quarter-0 prefill (slice 0): matmuls fill the
                        # exp-bound window; g spills f32 (tanh-argument
                        # precision), h spills bf16; silu conversion emitted
                        # last so it runs post-window on the idle scalar
                        # engine. ghq0 outlives the kqv pool close.
                        wgrW = d["wgT"].rearrange("(dt p) f -> p dt f", p=P)
                        whrW = d["whT"].rearrange("(dt p) f -> p dt f", p=P)
                        ghq0 = ghq0p.tile([P, 3, 512], F32R, tag="ghq0")
                        wf_cm = tc.tile_pool(name="wffn", bufs=3)
                        wfp = wf_cm.__enter__()
                        t0_cm = tc.tile_pool(name="t0p", bufs=1)
                        t0p = t0_cm.__enter__()
                        wgw_cm = tc.tile_pool(name="wgw", bufs=1)
                        wgwp = wgw_cm.__enter__()
                        x1n0 = x1ns[0]
                        g_sbs = []
                        h_sbs = []
                        for ht in range(3):
                            wgw = wgwp.tile([P, NDT, P], F32R, tag="wgw")
                            nc.sync.dma_start(
                                out=wgw, in_=wgrW[:, :, ht * P:(ht + 1) * P])
                            g_ps = gpsp.tile([P, 512], F32, tag="g")
                            for dt_ in range(NDT):
                                nc.tensor.matmul(
                                    g_ps, wgw[:, dt_, :], x1n0[:, dt_, :],
                                    start=(dt_ == 0), stop=(dt_ == NDT - 1))
                            g_sb = wfp.tile([P, 512], F32, tag="g_sb")
                            nc.vector.tensor_copy(g_sb, g_ps)
                            whw = wgwp.tile([P, NDT, P], F32R, tag="wgw")
                            nc.sync.dma_start(
                                out=whw, in_=whrW[:, :, ht * P:(ht + 1) * P])
                            h_ps = hpsp.tile([P, 512], F32, tag="h")
                            for dt_ in range(NDT):
                                nc.tensor.matmul(
                                    h_ps, whw[:, dt_, :], x1n0[:, dt_, :],
                                    start=(dt_ == 0), stop=(dt_ == NDT - 1))
                            h_sb = wfp.tile([P, 512], BF16, tag="h_sb")
                            nc.vector.tensor_copy(h_sb, h_ps)
                            g_sbs.append(g_sb)
                            h_sbs.append(h_sb)
                        for hl in range(3):
                            t_sb0 = t0p.tile([P, 512], F32, tag="t0")
                            nc.scalar.activation(t_sb0, g_sbs[hl], AF.Tanh,
                                                 scale=0.5)
                            nc.vector.scalar_tensor_tensor(
                                out=g_sbs[hl], in0=t_sb0, scalar=1.0,
                                in1=g_sbs[hl], op0=ALU.add, op1=ALU.mult)
                            nc.vector.scalar_tensor_tensor(
                                out=ghq0[:, hl, :], in0=g_sbs[hl], scalar=0.5,
                                in1=h_sbs[hl], op0=ALU.mult, op1=ALU.mult)
                        wgw_cm.__exit__(None, None, None)
                        t0_cm.__exit__(None, None, None)
                        wf_cm.__exit__(None, None, None)
        # kqv closed: FFN pools reuse its SBUF space

        # ---- phase 3: FFN (f32r), quarter-of-hidden, both slices ----
        with ExitStack() as fctx:
            fpp = fctx.enter_context(
                tc.tile_pool(name="fpp", bufs=2, space="PSUM"))
            gpsp2 = fctx.enter_context(
                tc.tile_pool(name="gps2", bufs=1, space="PSUM"))
            hpsp2 = fctx.enter_context(
                tc.tile_pool(name="hps2", bufs=1, space="PSUM"))
            ghp = fctx.enter_context(tc.tile_pool(name="ghq", bufs=2))
            tsp = fctx.enter_context(tc.tile_pool(name="tsb", bufs=3))
            finp = fctx.enter_context(tc.tile_pool(name="fin", bufs=2))
            wghp = fctx.enter_context(tc.tile_pool(name="wgh", bufs=2))
            wobp = fctx.enter_context(tc.tile_pool(name="wob", bufs=2))
            wgr = d["wgT"].rearrange("(dt p) f -> p dt f", p=P)
            whr = d["whT"].rearrange("(dt p) f -> p dt f", p=P)
            wor8 = d["woB"].rearrange("(ht p) f -> p ht f", p=P)
            NQ = NHT // 4
            for ns in range(2):
                qsl = slice(ns * 512, (ns + 1) * 512)
                x1n = x1ns[ns]
                for q4 in range(4):
                    ghq = ghp.tile([P, NQ, 512], F32R, tag="ghq")
                    for hl in range(NQ):
                        ht = q4 * NQ + hl
                        if ns == 0 and q4 == 0 and hl < 3:
                            continue
                        wg = wghp.tile([P, NDT, P], F32R, tag="wg")
                        wh = wghp.tile([P, NDT, P], F32R, tag="wh")
                        nc.sync.dma_start(
                            out=wg, in_=wgr[:, :, ht * P:(ht + 1) * P])
                        nc.sync.dma_start(
                            out=wh, in_=whr[:, :, ht * P:(ht + 1) * P])
                        if ht % 2 == 0:
                            g_ps = gpsp.tile([P, 512], F32, tag="g")
                            h_ps = hpsp.tile([P, 512], F32, tag="h")
                        else:
                            g_ps = gpsp2.tile([P, 512], F32, tag="g2")
                            h_ps = hpsp2.tile([P, 512], F32, tag="h2")
                        for dt_ in range(NDT):
                            nc.tensor.matmul(
                                g_ps, wg[:, dt_, :], x1n[:, dt_, :],
                                start=(dt_ == 0), stop=(dt_ == NDT - 1))
                        for dt_ in range(NDT):
                            nc.tensor.matmul(
                                h_ps, wh[:, dt_, :], x1n[:, dt_, :],
                                start=(dt_ == 0), stop=(dt_ == NDT - 1))
                        # silu(g)*h via tanh: t=tanh(g/2); gh=0.5*g*(1+t)*h
                        t_sb = tsp.tile([P, 512], F32, tag="tanh")
                        nc.scalar.activation(t_sb, g_ps, AF.Tanh, scale=0.5)
                        tmp = tsp.tile([P, 512], F32, tag="tmp")
                        nc.vector.scalar_tensor_tensor(
                            out=tmp, in0=t_sb, scalar=1.0, in1=g_ps,
                            op0=ALU.add, op1=ALU.mult)
                        nc.vector.scalar_tensor_tensor(
                            out=ghq[:, hl, :], in0=tmp, scalar=0.5,
                            in1=h_ps, op0=ALU.mult, op1=ALU.mult)
                    for fo in range(NDT):
                        wob = wobp.tile([P, NQ, P], F32R, tag="wob")
                        nc.sync.dma_start(
                            out=wob,
                            in_=wor8[:, q4 * NQ:(q4 + 1) * NQ,
                                     fo * P:(fo + 1) * P])
                        fp = fpp.tile([P, 512], F32, tag="fp")
                        for hl in range(NQ):
                            if ns == 0 and q4 == 0 and hl < 3:
                                rhs = ghq0[:, hl, :]
                            else:
                                rhs = ghq[:, hl, :]
                            nc.tensor.matmul(
                                fp, wob[:, hl, :], rhs,
                                start=(hl == 0), stop=(hl == NQ - 1))
                        if q4 < 3:
                            nc.vector.tensor_add(
                                x1T[:, fo, qsl], fp, x1T[:, fo, qsl])
                        else:
                            yt = finp.tile([P, 512], F32, tag="yt")
                            nc.vector.scalar_tensor_tensor(
                                out=yt, in0=fp,
                                scalar=bout_sb[:, fo:fo + 1],
                                in1=x1T[:, fo, qsl],
                                op0=ALU.add, op1=ALU.add)
                            nc.gpsimd.dma_start(
                                out=d["yT"][fo * P:(fo + 1) * P, qsl],
                                in_=yt)


_NC_CACHE = {}


def kernel(x, W_q, W_k, W_v, W_o, b_o, attn_norm_w, ffn_norm_w,
           W_gate, W_hidden, W_out, b_out):
    x = np.asarray(x, np.float32)
    f32 = lambda a: np.ascontiguousarray(np.asarray(a, np.float32))
    bf16 = lambda a: np.ascontiguousarray(
        np.asarray(a, np.float32).astype(ml_dtypes.bfloat16))
    w1 = np.asarray(attn_norm_w, np.float32)[:, None]
    w2 = np.asarray(ffn_norm_w, np.float32)[:, None]
    wqT = bf16(np.asarray(W_q, np.float32).T * w1 / np.sqrt(HD))
    wkT = bf16(np.asarray(W_k, np.float32).T * w1)
    wvT = bf16(np.asarray(W_v, np.float32).T * w1)
    woT = bf16(np.asarray(W_o, np.float32).T)
    def f22(a):
        b = np.ascontiguousarray(np.asarray(a, np.float32)).view(np.uint32)
        return ((b >> 10) << 10).view(np.float32)
    wgT = f22(np.asarray(W_gate, np.float32).T * w2)
    whT = f22(np.asarray(W_hidden, np.float32).T * w2)
    woB = f22(np.asarray(W_out, np.float32).T)
    bo_t = f32(np.asarray(b_o, np.float32).reshape(NDT, P).T)
    bout_t = f32(np.asarray(b_out, np.float32).reshape(NDT, P).T)

    if "nc" not in _NC_CACHE:
        _NC_CACHE["nc"] = build_nc()
    nc = _NC_CACHE["nc"]

    in_maps = []
    for c in range(8):
        b, half = c // 2, c % 2
        xb = x[b]
        if half:
            xb = np.concatenate([xb[LQ:], xb[:LQ]], axis=0)
        in_maps.append({
            "xT": np.ascontiguousarray(xb.T),
            "wqT": wqT, "wkT": wkT, "wvT": wvT, "woT": woT,
            "bo_t": bo_t, "wgT": wgT, "whT": whT, "woB": woB,
            "bout_t": bout_t,
        })
    global _LAST_IN_MAPS
    _LAST_IN_MAPS = in_maps
    res = bass_utils.run_bass_kernel_spmd(nc, in_maps, core_ids=list(range(8)))
    y = np.empty((4, L, D), np.float32)
    for c in range(8):
        b, half = c // 2, c % 2
        y[b, half * LQ:(half + 1) * LQ, :] = res.results[c]["yT"].T
    return y



# revision 12
# speedup vs baseline: 1.2099x; 1.2099x over previous
"""Dense transformer block (RMSNorm+MHA+residual, RMSNorm+SwiGLU+residual)
on 8 trn2 NeuronCores. Sharding: 2 cores per batch element; each core
computes the block output for 1024 of its batch's 2048 tokens, redundantly
computing K/V for the full sequence (keys are permutation invariant; each
core's x puts its own 1024 query tokens first). No inter-core communication.

fp8 (e4m3) DoubleRow rewrite: every large matmul runs fp8 with
MatmulPerfMode.DoubleRow (0.5 cycles/row, 256-deep contraction pairs),
cutting PE time ~4x vs bf16 per MAC. Numerics validated against the
reference in numpy (max abs err ~0.07 vs gate 0.109):
- attention path entirely 1-term fp8 (x, wq/wk/wv/wo, k/q/v, probs, attn)
  with power-of-2 scales; rmsnorm scales folded into PSUM evacuations.
- FFN gate/hidden: 2-term (flat-scale residual) fp8 weights x 1-term fp8
  x1n; out-proj: 3-pass (w8*gh8 + w8*ghr8 + wr8*gh8) with 2-term gh split
  computed on Pool (cast) + DVE (residual subtract).
- softmax exp split between ACT (table exp -> fp8) and DVE (Schraudolph
  uint8 bit-trick written through a bitcast into the same fp8 tile);
  softmax normalization cancels the bit-trick's systematic error.
- scores use a repartitioned K/Q layout [32, 2(hd-half), head, tokens]
  produced via a DRAM round-trip so DoubleRow can pair the two 32-feature
  halves of each 64-wide head.
- silu via tanh identity keeps ACT on the exp-compatible table set during
  the attention window; FFN(slice0) gate/hidden matmuls overlap the
  slice-1 attention window (PSUM budget: scores 2x2 + acc 1 + Wo/ss 1 +
  gate/hidden 1+1).
"""
import sys
from contextlib import ExitStack

import numpy as np

sys.path.insert(0, "/opt/trn_rl_repo")

import ml_dtypes  # noqa: E402
import concourse.bass as bass  # noqa: E402
from concourse import bacc  # noqa: E402
import concourse.tile as tile  # noqa: E402
from concourse import mybir  # noqa: E402
from concourse import bass_utils  # noqa: E402

P = 128
D = 1024          # d_model
L = 2048          # full seq per core (keys)
LQ = 1024         # query tokens per core
NH = 16
HD = 64
HID = 4096
EPS = 1e-6
NDT = D // P      # 8 feature tiles
NKT = L // P      # 16 key tiles
NHT = HID // P    # 32 hidden tiles
LN2 = float(np.log(2.0))

# power-of-2 fp8 scales (validated in acc_sim.py)
SX1 = 16.0        # x8 = fp8(x * SX1)
SWQ, SWK, SWV, SWO = 8192.0, 2048.0, 2048.0, 1024.0
SKQ, SQ2, SV, SA = 32.0, 256.0, 32.0, 1024.0
SX2, SWF, SGH = 16.0, 1024.0, 16.0
# Schraudolph exp on DVE: uint8 bits = st*K2B + BCONST, bitcast to e4m3
K2B = float(8.0 * np.log2(np.e) / (SKQ * SQ2))   # st = 8192 * s_true
BCONST = 55.5                                     # 7*8 + c_adj(-0.5)
EXP_DVE_MOD = 3   # every 3rd exp group goes to DVE (1/3 DVE, 2/3 ACT)

F32 = mybir.dt.float32
BF16 = mybir.dt.bfloat16
FP8 = mybir.dt.float8e4
U8 = mybir.dt.uint8
AF = mybir.ActivationFunctionType
ALU = mybir.AluOpType
DR = mybir.MatmulPerfMode.DoubleRow
E4 = ml_dtypes.float8_e4m3

SIM_TIME_NS = None


def build_nc():
    global SIM_TIME_NS
    nc = bacc.Bacc(None, target_bir_lowering=False)
    d = {}
    d["x8T"] = nc.dram_tensor("x8T", [D, L], FP8, kind="ExternalInput")
    d["xqT"] = nc.dram_tensor("xqT", [D, LQ], F32, kind="ExternalInput")
    d["wq8"] = nc.dram_tensor("wq8", [D, D], FP8, kind="ExternalInput")
    d["wk8"] = nc.dram_tensor("wk8", [D, D], FP8, kind="ExternalInput")
    d["wv8"] = nc.dram_tensor("wv8", [D, D], FP8, kind="ExternalInput")
    d["wo8"] = nc.dram_tensor("wo8", [D, D], FP8, kind="ExternalInput")
    d["wg8"] = nc.dram_tensor("wg8", [D, HID], FP8, kind="ExternalInput")
    d["wgr8"] = nc.dram_tensor("wgr8", [D, HID], FP8, kind="ExternalInput")
    d["wh8"] = nc.dram_tensor("wh8", [D, HID], FP8, kind="ExternalInput")
    d["whr8"] = nc.dram_tensor("whr8", [D, HID], FP8, kind="ExternalInput")
    d["wob8"] = nc.dram_tensor("wob8", [HID, D], FP8, kind="ExternalInput")
    d["wobr8"] = nc.dram_tensor("wobr8", [HID, D], FP8, kind="ExternalInput")
    d["bout_row"] = nc.dram_tensor("bout_row", [1, D], BF16, kind="ExternalInput")
    d["yT"] = nc.dram_tensor("yT", [D, LQ], F32, kind="ExternalOutput")

    with tile.TileContext(nc) as tc:
        _body(tc, nc, d)
        _, snap = tc.schedule_and_allocate()
        SIM_TIME_NS = snap.time
    nc.compile()
    return nc


def _body(tc, nc, d):
    x8Tr = d["x8T"].rearrange("(dt p) l -> p dt l", p=P)
    xqTr = d["xqT"].rearrange("(dt p) l -> p dt l", p=P)

    with ExitStack() as pp_ctx:
        pp = pp_ctx.enter_context(tc.tile_pool(name="persist", bufs=1))
        eps_t = pp.tile([1, 1], F32, tag="eps")
        bm10 = pp.tile([1, 1], F32, tag="bm10")
        bm9 = pp.tile([1, 1], F32, tag="bm9")
        bp4 = pp.tile([1, 1], F32, tag="bp4")
        ones_col = pp.tile([P, 1], BF16, tag="ones")
        ones_row = pp.tile([1, 512], BF16, tag="onesr")
        bout_sb = pp.tile([1, D], BF16, tag="bout")
        bconst = pp.tile([P, 2, 512], BF16, tag="bconst")
        x1T = pp.tile([P, NDT, LQ], F32, tag="x1T")
        x1n_o = pp_ctx.enter_context(tc.tile_pool(name="x1n", bufs=2))
        nc.vector.memset(eps_t, EPS)
        nc.vector.memset(bm10, -10.0 * LN2)
        nc.vector.memset(bm9, -9.0 * LN2)
        nc.vector.memset(bp4, 4.0 * LN2)
        nc.vector.memset(ones_col, 1.0)
        nc.vector.memset(ones_row, 1.0)
        nc.vector.memset(bconst, BCONST)
        nc.sync.dma_start(out=bout_sb, in_=d["bout_row"][:, :])
        x1ns = []
        ghq_sets = []

        # FFN PSUM pools allocated early (shared across overlap window)
        gpsp = pp_ctx.enter_context(tc.tile_pool(name="gps", bufs=1, space="PSUM"))
        hpsp = pp_ctx.enter_context(tc.tile_pool(name="hps", bufs=1, space="PSUM"))
        # gh tiles for slice-0 FFN prefill (persist until out-proj)
        ghp = pp_ctx.enter_context(tc.tile_pool(name="ghq", bufs=1))

        wgr_ = d["wg8"].rearrange("(dt p) f -> p dt f", p=P)
        wgrr_ = d["wgr8"].rearrange("(dt p) f -> p dt f", p=P)
        whr_ = d["wh8"].rearrange("(dt p) f -> p dt f", p=P)
        whrr_ = d["whr8"].rearrange("(dt p) f -> p dt f", p=P)

        def ffn_gh(ns, ghq8, ghqr8, wfp, tsp, gbp, pools):
            """gate/hidden + silu chain for one slice -> gh8/ghr8 tiles."""
            x1n = x1ns[ns]
            for ht in range(NHT):
                gpool, hpool = pools[ht % len(pools)]
                wg = wfp.tile([P, NDT, P], FP8, tag="wg")
                nc.sync.dma_start(out=wg, in_=wgr_[:, :, ht * P:(ht + 1) * P])
                wgr = wfp.tile([P, NDT, P], FP8, tag="wgr")
                nc.sync.dma_start(out=wgr, in_=wgrr_[:, :, ht * P:(ht + 1) * P])
                wh = wfp.tile([P, NDT, P], FP8, tag="wh")
                nc.sync.dma_start(out=wh, in_=whr_[:, :, ht * P:(ht + 1) * P])
                whr = wfp.tile([P, NDT, P], FP8, tag="whr")
                nc.sync.dma_start(out=whr, in_=whrr_[:, :, ht * P:(ht + 1) * P])
                g_ps = gpool.tile([P, 512], F32, tag="g")
                for dp in range(NDT // 2):
                    s2 = slice(2 * dp, 2 * dp + 2)
                    nc.tensor.matmul(g_ps, wg[:, s2, :], x1n[:, s2, :],
                                     start=(dp == 0), stop=False, perf_mode=DR)
                for dp in range(NDT // 2):
                    s2 = slice(2 * dp, 2 * dp + 2)
                    nc.tensor.matmul(g_ps, wgr[:, s2, :], x1n[:, s2, :],
                                     start=False, stop=(dp == NDT // 2 - 1),
                                     perf_mode=DR)
                h_ps = hpool.tile([P, 512], F32, tag="h")
                for dp in range(NDT // 2):
                    s2 = slice(2 * dp, 2 * dp + 2)
                    nc.tensor.matmul(h_ps, wh[:, s2, :], x1n[:, s2, :],
                                     start=(dp == 0), stop=False, perf_mode=DR)
                for dp in range(NDT // 2):
                    s2 = slice(2 * dp, 2 * dp + 2)
                    nc.tensor.matmul(h_ps, whr[:, s2, :], x1n[:, s2, :],
                                     start=False, stop=(dp == NDT // 2 - 1),
                                     perf_mode=DR)
                # silu(g)*h via tanh (stays on exp-compatible ACT table):
                # t = tanh(G/2); gh = 0.5*G*(1+t)*H, scaled to SGH*gh in bf16
                t_sb = tsp.tile([P, 512], F32, tag="tanh")
                nc.scalar.activation(t_sb, g_ps, AF.Tanh, scale=2.0 ** -15)
                tmp = tsp.tile([P, 512], F32, tag="tmp")
                nc.vector.scalar_tensor_tensor(
                    out=tmp, in0=t_sb, scalar=1.0, in1=g_ps,
                    op0=ALU.add, op1=ALU.mult)
                gh_bf = gbp.tile([P, 512], BF16, tag="ghbf")
                nc.vector.scalar_tensor_tensor(
                    out=gh_bf, in0=tmp, scalar=2.0 ** -25, in1=h_ps,
                    op0=ALU.mult, op1=ALU.mult)
                # 2-term split: gh8 = fp8(gh_bf) on Pool; ghr8 = gh_bf - gh8
                nc.gpsimd.tensor_scalar(out=ghq8[:, ht, :], in0=gh_bf,
                                        scalar1=1.0, scalar2=None,
                                        op0=ALU.mult)
                nc.vector.scalar_tensor_tensor(
                    out=ghqr8[:, ht, :], in0=ghq8[:, ht, :], scalar=-1.0,
                    in1=gh_bf, op0=ALU.mult, op1=ALU.add)

        def ffn_out(ns, ghq8, ghqr8, fpp, wop, finp):
            """out-projection 3-pass + bias + evac for one slice."""
            qsl = slice(ns * 512, (ns + 1) * 512)
            wor_ = d["wob8"].rearrange("(ht p) f -> p ht f", p=P)
            worr_ = d["wobr8"].rearrange("(ht p) f -> p ht f", p=P)
            for fo in range(NDT):
                wob = wop.tile([P, NHT, P], FP8, tag="wob")
                nc.sync.dma_start(out=wob, in_=wor_[:, :, fo * P:(fo + 1) * P])
                wobr = wop.tile([P, NHT, P], FP8, tag="wobr")
                nc.sync.dma_start(out=wobr, in_=worr_[:, :, fo * P:(fo + 1) * P])
                fp = fpp.tile([P, 512], F32, tag="fp")
                for hp in range(NHT // 2):
                    s2 = slice(2 * hp, 2 * hp + 2)
                    nc.tensor.matmul(fp, wob[:, s2, :], ghq8[:, s2, :],
                                     start=(hp == 0), stop=False, perf_mode=DR)
                for hp in range(NHT // 2):
                    s2 = slice(2 * hp, 2 * hp + 2)
                    nc.tensor.matmul(fp, wob[:, s2, :], ghqr8[:, s2, :],
                                     start=False, stop=False, perf_mode=DR)
                for hp in range(NHT // 2):
                    s2 = slice(2 * hp, 2 * hp + 2)
                    nc.tensor.matmul(fp, wobr[:, s2, :], ghq8[:, s2, :],
                                     start=False, stop=False, perf_mode=DR)
                # + b_out (scaled 2^14) via rank-1 bf16 matmul
                nc.tensor.matmul(fp, bout_sb[:, fo * P:(fo + 1) * P],
                                 ones_row, start=False, stop=True)
                yt = finp.tile([P, 512], F32, tag="yt")
                nc.vector.scalar_tensor_tensor(
                    out=yt, in0=fp, scalar=2.0 ** -14, in1=x1T[:, fo, qsl],
                    op0=ALU.mult, op1=ALU.add)
                nc.gpsimd.dma_start(out=d["yT"][fo * P:(fo + 1) * P, qsl],
                                    in_=yt)

        with ExitStack() as actx:
            ap = actx.enter_context(tc.tile_pool(name="attn", bufs=1))
            vt = ap.tile([P, NKT, NH, HD + 1], FP8, tag="vt")
            kdrp = actx.enter_context(tc.tile_pool(name="kdr", bufs=1, space="DRAM"))
            kdr = kdrp.tile([D, L], FP8, tag="kdr")
            qdr = kdrp.tile([D, LQ], FP8, tag="qdr")
            kthp = actx.enter_context(tc.tile_pool(name="kth", bufs=3))
            attnT = ap.tile([P, NDT, 512], FP8, tag="attnT")
            nc.vector.memset(vt[:, :, :, HD:HD + 1], SV / SA)

            # ---- P0: load x8, rmsnorm stats ----
            with ExitStack() as pctx:
                xp = pctx.enter_context(tc.tile_pool(name="xp", bufs=1))
                n1p = pctx.enter_context(tc.tile_pool(name="n1", bufs=3))
                bcp = pctx.enter_context(tc.tile_pool(name="bc1", bufs=2))
                bcP = pctx.enter_context(tc.tile_pool(name="bcP", bufs=1))
                rscp = pctx.enter_context(
                    tc.tile_pool(name="rsc", bufs=1, space="DRAM"))
                ssp = pctx.enter_context(
                    tc.tile_pool(name="ss1", bufs=2, space="PSUM"))
                prp = pctx.enter_context(
                    tc.tile_pool(name="proj", bufs=4, space="PSUM"))

                x8 = xp.tile([P, NDT, L], FP8, tag="x8")
                bck_all = bcP.tile([P, L // 512, 512], F32, tag="bck")
                bcq_all = bcP.tile([P, LQ // 512, 512], F32, tag="bcq")
                for ls in range(L // 512):
                    sl = slice(ls * 512, (ls + 1) * 512)
                    nc.gpsimd.dma_start(out=x8[:, :, sl], in_=x8Tr[:, :, sl])
                rsc = rscp.tile([L], F32, tag="rsc")
                for ls in range(L // 512):
                    sl = slice(ls * 512, (ls + 1) * 512)
                    ss_ps = ssp.tile([1, 512], F32, tag="ss")
                    for dt_ in range(NDT):
                        sq = n1p.tile([P, 512], BF16, tag="sq")
                        nc.gpsimd.tensor_mul(sq, x8[:, dt_, sl], x8[:, dt_, sl])
                        nc.tensor.matmul(ss_ps, ones_col, sq,
                                         start=(dt_ == 0), stop=(dt_ == NDT - 1))
                    lnr = bcp.tile([1, 512], F32, tag="lnr")
                    nc.scalar.activation(lnr, ss_ps, AF.Ln,
                                         bias=eps_t, scale=2.0 ** -18)
                    # rr_k = rr * 2^-10  (K evac, V evac);  rr_q = rr * 2^-9
                    rrk = bcp.tile([1, 512], F32, tag="rrk")
                    nc.scalar.activation(rrk, lnr, AF.Exp, scale=-0.5,
                                         bias=bm10)
                    nc.gpsimd.partition_broadcast(bck_all[:, ls, :], rrk)
                    nc.sync.dma_start(out=rsc[sl], in_=rrk)
                    if ls < LQ // 512:
                        rrq = bcp.tile([1, 512], F32, tag="rrq")
                        nc.scalar.activation(rrq, lnr, AF.Exp, scale=-0.5,
                                             bias=bm9)
                        nc.gpsimd.partition_broadcast(bcq_all[:, ls, :], rrq)
                rcol = bcP.tile([P, NKT], F32, tag="rcol")
                nc.sync.dma_start(out=rcol,
                                  in_=rsc.rearrange("(tt p) -> p tt", p=P))

                # ---- P1: K/Q/V projections (fp8 DR) ----
                with tc.tile_pool(name="wblk", bufs=2) as wp, \
                     tc.tile_pool(name="kmid", bufs=3) as kmp:
                    for (w_d, dr_t, ntok, bcl) in (
                            (d["wk8"], kdr, L, bck_all),
                            (d["wq8"], qdr, LQ, bcq_all)):
                        wr = w_d.rearrange("(dt p) f -> p dt f", p=P)
                        for ft in range(NDT):
                            wblk = wp.tile([P, NDT, P], FP8, tag="wblk")
                            nc.sync.dma_start(
                                out=wblk, in_=wr[:, :, ft * P:(ft + 1) * P])
                            for ks in range(ntok // 512):
                                sl = slice(ks * 512, (ks + 1) * 512)
                                ps = prp.tile([P, 512], F32, tag="pp")
                                for dp in range(NDT // 2):
                                    s2 = slice(2 * dp, 2 * dp + 2)
                                    nc.tensor.matmul(
                                        ps, wblk[:, s2, :], x8[:, s2, sl],
                                        start=(dp == 0),
                                        stop=(dp == NDT // 2 - 1), perf_mode=DR)
                                km = kmp.tile([P, 512], FP8, tag="km")
                                nc.vector.tensor_mul(km, ps, bcl[:, ks, :])
                                nc.sync.dma_start(
                                    out=dr_t[ft * P:(ft + 1) * P, sl], in_=km)
                    # V: token-major psum, ACT evac with per-token scale
                    wvr = d["wv8"].rearrange("(dt p) f -> p dt f", p=P)
                    for hf in range(2):
                        wv = wp.tile([P, NDT, 512], FP8, tag="wblk")
                        nc.sync.dma_start(
                            out=wv, in_=wvr[:, :, hf * 512:(hf + 1) * 512])
                        for tt in range(NKT):
                            ps = prp.tile([P, 512], F32, tag="pp")
                            for dp in range(NDT // 2):
                                s2 = slice(2 * dp, 2 * dp + 2)
                                nc.tensor.matmul(
                                    ps, x8[:, s2, tt * P:(tt + 1) * P],
                                    wv[:, s2, :],
                                    start=(dp == 0),
                                    stop=(dp == NDT // 2 - 1), perf_mode=DR)
                            nc.scalar.activation(
                                vt[:, tt, hf * 8:(hf + 1) * 8, 0:HD],
                                ps.rearrange("p (h e) -> p h e", h=8),
                                AF.Copy, scale=rcol[:, tt:tt + 1])

            # repartitioned K/Q views for DoubleRow scores (streamed per head):
            # feature f = dt*128 + h2*64 + hi*32 + lo ; head = 2*dt + h2
            kre = kdr.rearrange(
                "(dt h2 hi lo) k -> lo hi (dt h2) k",
                dt=NDT, h2=2, hi=2, lo=32)
            qre = qdr.rearrange(
                "(dt h2 hi lo) k -> lo hi (dt h2) k",
                dt=NDT, h2=2, hi=2, lo=32)

            # ---- P2: attention per 512-query slice ----
            with ExitStack() as ectx:
                ptp = ectx.enter_context(tc.tile_pool(name="pt", bufs=3))
                smp = ectx.enter_context(tc.tile_pool(name="sm", bufs=2))
                rbp = ectx.enter_context(tc.tile_pool(name="rb", bufs=2))
                xqp = ectx.enter_context(tc.tile_pool(name="xq", bufs=3))
                sqp = ectx.enter_context(tc.tile_pool(name="sq2p", bufs=2))
                wop = ectx.enter_context(tc.tile_pool(name="wo", bufs=2))
                tsp0 = ectx.enter_context(tc.tile_pool(name="ts0", bufs=3))
                gbp0 = ectx.enter_context(tc.tile_pool(name="gb0", bufs=4))
                wfp0 = ectx.enter_context(tc.tile_pool(name="wf0", bufs=2))
                stp = ectx.enter_context(
                    tc.tile_pool(name="st", bufs=2, space="PSUM"))
                accp = ectx.enter_context(
                    tc.tile_pool(name="acc", bufs=1, space="PSUM"))
                mpsp = ectx.enter_context(
                    tc.tile_pool(name="mps", bufs=1, space="PSUM"))
                gcount = 0
                for ns in range(2):
                    qsl = slice(ns * 512, (ns + 1) * 512)
                    for h in range(NH):
                        dt_ = h // 2
                        r0 = (h % 2) * HD
                        kth = kthp.tile([32, 2, L], FP8, tag="kth")
                        nc.sync.dma_start(out=kth, in_=kre[:, :, h, :])
                        qth = kthp.tile([32, 2, 512], FP8, tag="qth")
                        nc.sync.dma_start(out=qth, in_=qre[:, :, h, qsl])
                        acc = accp.tile([HD + 1, 512], F32, tag="acc")
                        for g in range(NKT // 2):
                            st = stp.tile([P, 2, 512], F32, tag="st")
                            for j in range(2):
                                kt = 2 * g + j
                                nc.tensor.matmul(
                                    st[:, j, :],
                                    kth[:, :, kt * P:(kt + 1) * P],
                                    qth,
                                    start=True, stop=True, perf_mode=DR)
                            pt = ptp.tile([P, 2, 512], FP8, tag="pt")
                            if gcount % EXP_DVE_MOD == EXP_DVE_MOD - 1:
                                nc.vector.scalar_tensor_tensor(
                                    out=pt.bitcast(U8), in0=st, scalar=K2B,
                                    in1=bconst, op0=ALU.mult, op1=ALU.add)
                            else:
                                nc.scalar.activation(pt, st, AF.Exp,
                                                     scale=1.0 / (SKQ * SQ2))
                            gcount += 1
                            nc.tensor.matmul(
                                acc, vt[:, 2 * g:2 * g + 2, h, :], pt,
                                start=(g == 0), stop=(g == NKT // 2 - 1),
                                perf_mode=DR)
                        rrow = smp.tile([1, 512], F32, tag="row")
                        nc.vector.reciprocal(rrow, acc[HD:HD + 1, :])
                        rb = rbp.tile([HD, 512], F32, tag="rb")
                        nc.gpsimd.partition_broadcast(rb, rrow)
                        nc.vector.tensor_mul(
                            attnT[r0:r0 + HD, dt_, :], acc[0:HD, :], rb)

                    # Wo projection (fp8 DR) + residual -> x1T
                    wor = d["wo8"].rearrange("(dt p) f -> p dt f", p=P)
                    for ft in range(NDT):
                        wblk = wop.tile([P, NDT, P], FP8, tag="woblk")
                        nc.sync.dma_start(
                            out=wblk, in_=wor[:, :, ft * P:(ft + 1) * P])
                        xq = xqp.tile([P, 512], F32, tag="xq")
                        nc.gpsimd.dma_start(out=xq, in_=xqTr[:, ft, qsl])
                        ps = mpsp.tile([P, 512], F32, tag="m")
                        for dp in range(NDT // 2):
                            s2 = slice(2 * dp, 2 * dp + 2)
                            nc.tensor.matmul(
                                ps, wblk[:, s2, :], attnT[:, s2, :],
                                start=(dp == 0), stop=(dp == NDT // 2 - 1),
                                perf_mode=DR)
                        nc.vector.scalar_tensor_tensor(
                            out=x1T[:, ft, qsl], in0=ps, scalar=2.0 ** -20,
                            in1=xq, op0=ALU.mult, op1=ALU.add)

                    # rmsnorm2 -> x1n fp8 (scale SX2 folded into exp bias)
                    ss2 = mpsp.tile([1, 512], F32, tag="m")
                    for dt_ in range(NDT):
                        sq2 = sqp.tile([P, 512], BF16, tag="sq2")
                        nc.gpsimd.tensor_mul(sq2, x1T[:, dt_, qsl],
                                             x1T[:, dt_, qsl])
                        nc.tensor.matmul(ss2, ones_col, sq2,
                                         start=(dt_ == 0), stop=(dt_ == NDT - 1))
                    ln2 = smp.tile([1, 512], F32, tag="row")
                    nc.scalar.activation(ln2, ss2, AF.Ln, bias=eps_t,
                                         scale=1.0 / D)
                    rr2 = smp.tile([1, 512], F32, tag="row")
                    nc.scalar.activation(rr2, ln2, AF.Exp, scale=-0.5,
                                         bias=bp4)
                    bc2 = rbp.tile([P, 512], F32, tag="rb2")
                    nc.gpsimd.partition_broadcast(bc2, rr2)
                    x1n = x1n_o.tile([P, NDT, 512], FP8, tag="x1n")
                    for dt_ in range(NDT):
                        nc.gpsimd.tensor_mul(x1n[:, dt_, :], x1T[:, dt_, qsl],
                                             bc2)
                    x1ns.append(x1n)
                    if ns == 0:
                        # FFN slice-0 gate/hidden overlaps slice-1 attention
                        ghq8 = ghp.tile([P, NHT, 512], FP8, tag="gh8")
                        ghqr8 = ghp.tile([P, NHT, 512], FP8, tag="ghr8")
                        ghq_sets.append((ghq8, ghqr8))
                        ffn_gh(0, ghq8, ghqr8, wfp0, tsp0, gbp0,
                               [(gpsp, hpsp)])
            # attention pools closed: PSUM st/acc/mps freed

        # ---- P3: out-proj(0), gate/hidden(1), out-proj(1) ----
        with ExitStack() as fctx:
            fpp = fctx.enter_context(
                tc.tile_pool(name="fpp", bufs=2, space="PSUM"))
            gpsp2 = fctx.enter_context(
                tc.tile_pool(name="gps2", bufs=1, space="PSUM"))
            hpsp2 = fctx.enter_context(
                tc.tile_pool(name="hps2", bufs=1, space="PSUM"))
            tsp = fctx.enter_context(tc.tile_pool(name="tsb", bufs=3))
            gbp = fctx.enter_context(tc.tile_pool(name="gb1", bufs=4))
            wfp = fctx.enter_context(tc.tile_pool(name="wffn", bufs=2))
            wop2 = fctx.enter_context(tc.tile_pool(name="wob2", bufs=2))
            finp = fctx.enter_context(tc.tile_pool(name="fin", bufs=2))

            ghq8, ghqr8 = ghq_sets[0]
            ffn_out(0, ghq8, ghqr8, fpp, wop2, finp)
            ghq8b = ghp.tile([P, NHT, 512], FP8, tag="gh8")
            ghqr8b = ghp.tile([P, NHT, 512], FP8, tag="ghr8")
            ffn_gh(1, ghq8b, ghqr8b, wfp, tsp, gbp,
                   [(gpsp, hpsp), (gpsp2, hpsp2)])
            ffn_out(1, ghq8b, ghqr8b, fpp, wop2, finp)


_NC_CACHE = {}


def kernel(x, W_q, W_k, W_v, W_o, b_o, attn_norm_w, ffn_norm_w,
           W_gate, W_hidden, W_out, b_out):
    x = np.asarray(x, np.float32)
    w1 = np.asarray(attn_norm_w, np.float32)[:, None]
    w2 = np.asarray(ffn_norm_w, np.float32)[:, None]

    def q8(a, sc):
        y = np.ascontiguousarray(np.asarray(a, np.float32)) * sc
        return y.astype(E4)

    def q8pair(a, sc):
        y = np.ascontiguousarray(np.asarray(a, np.float32)) * sc
        a1 = y.astype(E4)
        r = (y - a1.astype(np.float32)).astype(E4)
        return a1, r

    wq8 = q8(np.asarray(W_q, np.float32).T * w1 / np.sqrt(HD), SWQ)
    wk8 = q8(np.asarray(W_k, np.float32).T * w1, SWK)
    wv8 = q8(np.asarray(W_v, np.float32).T * w1, SWV)
    wo8 = q8(np.asarray(W_o, np.float32).T, SWO)
    wg8, wgr8 = q8pair(np.asarray(W_gate, np.float32).T * w2, SWF)
    wh8, whr8 = q8pair(np.asarray(W_hidden, np.float32).T * w2, SWF)
    wob8, wobr8 = q8pair(np.asarray(W_out, np.float32).T, SWF)
    bout_row = np.ascontiguousarray(
        (np.asarray(b_out, np.float32) * (SWF * SGH))[None, :]
    ).astype(ml_dtypes.bfloat16)
    bo = np.asarray(b_o, np.float32)

    if "nc" not in _NC_CACHE:
        _NC_CACHE["nc"] = build_nc()
    nc = _NC_CACHE["nc"]

    in_maps = []
    for c in range(8):
        b, half = c // 2, c % 2
        xb = x[b]
        if half:
            xb = np.concatenate([xb[LQ:], xb[:LQ]], axis=0)
        in_maps.append({
            "x8T": np.ascontiguousarray((xb.T * SX1)).astype(E4),
            "xqT": np.ascontiguousarray(xb[:LQ].T + bo[:, None]),
            "wq8": wq8, "wk8": wk8, "wv8": wv8, "wo8": wo8,
            "wg8": wg8, "wgr8": wgr8, "wh8": wh8, "whr8": whr8,
            "wob8": wob8, "wobr8": wobr8, "bout_row": bout_row,
        })
    res = bass_utils.run_bass_kernel_spmd(nc, in_maps, core_ids=list(range(8)))
    y = np.empty((4, L, D), np.float32)
    for c in range(8):
        b, half = c // 2, c % 2
        y[b, half * LQ:(half + 1) * LQ, :] = res.results[c]["yT"].T
    return y


# revision 15
# speedup vs baseline: 1.4191x; 1.1728x over previous
"""Dense transformer block (RMSNorm+MHA+residual, RMSNorm+SwiGLU+residual)
on 8 trn2 NeuronCores. Sharding: 2 cores per batch element; each core
computes the block output for 1024 of its batch's 2048 tokens, redundantly
computing K/V for the full sequence (keys are permutation invariant; each
core's x puts its own 1024 query tokens first). No inter-core communication.

fp8 (e4m3) DoubleRow rewrite: every large matmul runs fp8 with
MatmulPerfMode.DoubleRow (0.5 cycles/row, 256-deep contraction pairs),
cutting PE time ~4x vs bf16 per MAC. Numerics validated against the
reference in numpy (max abs err ~0.07 vs gate 0.109):
- attention path entirely 1-term fp8 (x, wq/wk/wv/wo, k/q/v, probs, attn)
  with power-of-2 scales; rmsnorm scales folded into PSUM evacuations.
- FFN gate/hidden: 2-term (flat-scale residual) fp8 weights x 1-term fp8
  x1n; out-proj: 3-pass (w8*gh8 + w8*ghr8 + wr8*gh8) with 2-term gh split
  computed on Pool (cast) + DVE (residual subtract).
- softmax exp split between ACT (table exp -> fp8) and DVE (Schraudolph
  uint8 bit-trick written through a bitcast into the same fp8 tile);
  softmax normalization cancels the bit-trick's systematic error.
- scores use a repartitioned K/Q layout [32, 2(hd-half), head, tokens]
  produced via a DRAM round-trip so DoubleRow can pair the two 32-feature
  halves of each 64-wide head.
- silu via tanh identity keeps ACT on the exp-compatible table set during
  the attention window; FFN(slice0) gate/hidden matmuls overlap the
  slice-1 attention window (PSUM budget: scores 2x2 + acc 1 + Wo/ss 1 +
  gate/hidden 1+1).
"""
import sys
from contextlib import ExitStack

import numpy as np

sys.path.insert(0, "/opt/trn_rl_repo")

import ml_dtypes  # noqa: E402
import concourse.bass as bass  # noqa: E402
from concourse import bacc  # noqa: E402
import concourse.tile as tile  # noqa: E402
from concourse import mybir  # noqa: E402
from concourse import bass_utils  # noqa: E402

P = 128
D = 1024          # d_model
L = 2048          # full seq per core (keys)
LQ = 1024         # query tokens per core
NH = 16
HD = 64
HID = 4096
EPS = 1e-6
NDT = D // P      # 8 feature tiles
NKT = L // P      # 16 key tiles
NHT = HID // P    # 32 hidden tiles
LN2 = float(np.log(2.0))

# power-of-2 fp8 scales (validated in acc_sim.py)
SX1 = 16.0        # x8 = fp8(x * SX1)
SWQ, SWK, SWV, SWO = 8192.0, 2048.0, 2048.0, 1024.0
SKQ, SQ2, SV, SA = 32.0, 256.0, 32.0, 1024.0
SX2, SWF, SGH = 16.0, 1024.0, 16.0
# Schraudolph exp on DVE: uint8 bits = st*K2B + BCONST, bitcast to e4m3
K2B = float(8.0 * np.log2(np.e) / (SKQ * SQ2))   # st = 8192 * s_true
BCONST = 55.5                                     # 7*8 + c_adj(-0.5)
EXP_DVE_MOD = 3   # every 3rd exp group goes to DVE (1/3 DVE, 2/3 ACT)

F32 = mybir.dt.float32
BF16 = mybir.dt.bfloat16
FP8 = mybir.dt.float8e4
U8 = mybir.dt.uint8
AF = mybir.ActivationFunctionType
ALU = mybir.AluOpType
DR = mybir.MatmulPerfMode.DoubleRow
E4 = ml_dtypes.float8_e4m3

SIM_TIME_NS = None


def build_nc():
    global SIM_TIME_NS
    nc = bacc.Bacc(None, target_bir_lowering=False)
    d = {}
    d["x8T"] = nc.dram_tensor("x8T", [D, L], FP8, kind="ExternalInput")
    d["xqT"] = nc.dram_tensor("xqT", [D, LQ], F32, kind="ExternalInput")
    d["wq8"] = nc.dram_tensor("wq8", [D, D], FP8, kind="ExternalInput")
    d["wk8"] = nc.dram_tensor("wk8", [D, D], FP8, kind="ExternalInput")
    d["wv8"] = nc.dram_tensor("wv8", [D, D], FP8, kind="ExternalInput")
    d["wo8"] = nc.dram_tensor("wo8", [D, D], FP8, kind="ExternalInput")
    d["wg8"] = nc.dram_tensor("wg8", [D, HID], FP8, kind="ExternalInput")
    d["wgr8"] = nc.dram_tensor("wgr8", [D, HID], FP8, kind="ExternalInput")
    d["wh8"] = nc.dram_tensor("wh8", [D, HID], FP8, kind="ExternalInput")
    d["whr8"] = nc.dram_tensor("whr8", [D, HID], FP8, kind="ExternalInput")
    d["wob8"] = nc.dram_tensor("wob8", [HID, D], FP8, kind="ExternalInput")
    d["wobr8"] = nc.dram_tensor("wobr8", [HID, D], FP8, kind="ExternalInput")
    d["bout_row"] = nc.dram_tensor("bout_row", [1, D], BF16, kind="ExternalInput")
    d["yT"] = nc.dram_tensor("yT", [D, LQ], F32, kind="ExternalOutput")

    with tile.TileContext(nc) as tc:
        _body(tc, nc, d)
        _, snap = tc.schedule_and_allocate()
        SIM_TIME_NS = snap.time
    nc.compile()
    return nc


def _body(tc, nc, d):
    x8Tr = d["x8T"].rearrange("(dt p) l -> p dt l", p=P)
    xqTr = d["xqT"].rearrange("(dt p) l -> p dt l", p=P)

    with ExitStack() as pp_ctx:
        pp = pp_ctx.enter_context(tc.tile_pool(name="persist", bufs=1))
        eps_t = pp.tile([1, 1], F32, tag="eps")
        bm10 = pp.tile([1, 1], F32, tag="bm10")
        bm9 = pp.tile([1, 1], F32, tag="bm9")
        bp4 = pp.tile([1, 1], F32, tag="bp4")
        ones_col = pp.tile([P, 1], BF16, tag="ones")
        ones_row = pp.tile([1, 512], BF16, tag="onesr")
        bout_sb = pp.tile([1, D], BF16, tag="bout")
        bconst = pp.tile([P, 2, 512], BF16, tag="bconst")
        x1T = pp.tile([P, NDT, LQ], F32, tag="x1T")
        x1n_o = pp_ctx.enter_context(tc.tile_pool(name="x1n", bufs=2))
        nc.vector.memset(eps_t, EPS)
        nc.vector.memset(bm10, -10.0 * LN2)
        nc.vector.memset(bm9, -9.0 * LN2)
        nc.vector.memset(bp4, 4.0 * LN2)
        nc.vector.memset(ones_col, 1.0)
        nc.vector.memset(ones_row, 1.0)
        nc.vector.memset(bconst, BCONST)
        nc.sync.dma_start(out=bout_sb, in_=d["bout_row"][:, :])
        x1ns = []
        ghq_sets = []

        # FFN PSUM pools allocated early (shared across overlap window)
        gpsp = pp_ctx.enter_context(tc.tile_pool(name="gps", bufs=1, space="PSUM"))
        hpsp = pp_ctx.enter_context(tc.tile_pool(name="hps", bufs=1, space="PSUM"))
        # gh tiles for slice-0 FFN prefill (persist until out-proj)
        ghp = pp_ctx.enter_context(tc.tile_pool(name="ghq", bufs=1))

        wgr_ = d["wg8"].rearrange("(dt p) f -> p dt f", p=P)
        wgrr_ = d["wgr8"].rearrange("(dt p) f -> p dt f", p=P)
        whr_ = d["wh8"].rearrange("(dt p) f -> p dt f", p=P)
        whrr_ = d["whr8"].rearrange("(dt p) f -> p dt f", p=P)

        def ffn_gh(ns, ghq8, ghqr8, wfp, tsp, gbp, pools):
            """gate/hidden + silu chain for one slice -> gh8/ghr8 tiles."""
            x1n = x1ns[ns]
            for ht in range(NHT):
                gpool, hpool = pools[ht % len(pools)]
                wg = wfp.tile([P, NDT, P], FP8, tag="wg")
                nc.sync.dma_start(out=wg, in_=wgr_[:, :, ht * P:(ht + 1) * P])
                wgr = wfp.tile([P, NDT, P], FP8, tag="wgr")
                nc.scalar.dma_start(out=wgr, in_=wgrr_[:, :, ht * P:(ht + 1) * P])
                wh = wfp.tile([P, NDT, P], FP8, tag="wh")
                nc.gpsimd.dma_start(out=wh, in_=whr_[:, :, ht * P:(ht + 1) * P])
                whr = wfp.tile([P, NDT, P], FP8, tag="whr")
                nc.sync.dma_start(out=whr, in_=whrr_[:, :, ht * P:(ht + 1) * P])
                g_ps = gpool.tile([P, 512], F32, tag="g")
                for dp in range(NDT // 2):
                    s2 = slice(2 * dp, 2 * dp + 2)
                    nc.tensor.matmul(g_ps, wg[:, s2, :], x1n[:, s2, :],
                                     start=(dp == 0), stop=False, perf_mode=DR)
                for dp in range(NDT // 2):
                    s2 = slice(2 * dp, 2 * dp + 2)
                    nc.tensor.matmul(g_ps, wgr[:, s2, :], x1n[:, s2, :],
                                     start=False, stop=(dp == NDT // 2 - 1),
                                     perf_mode=DR)
                h_ps = hpool.tile([P, 512], F32, tag="h")
                for dp in range(NDT // 2):
                    s2 = slice(2 * dp, 2 * dp + 2)
                    nc.tensor.matmul(h_ps, wh[:, s2, :], x1n[:, s2, :],
                                     start=(dp == 0), stop=False, perf_mode=DR)
                for dp in range(NDT // 2):
                    s2 = slice(2 * dp, 2 * dp + 2)
                    nc.tensor.matmul(h_ps, whr[:, s2, :], x1n[:, s2, :],
                                     start=False, stop=(dp == NDT // 2 - 1),
                                     perf_mode=DR)
                # silu(g)*h via tanh (stays on exp-compatible ACT table):
                # t = tanh(G/2); gh = 0.5*G*(1+t)*H, scaled to SGH*gh in bf16
                t_sb = tsp.tile([P, 512], F32, tag="tanh")
                nc.scalar.activation(t_sb, g_ps, AF.Tanh, scale=2.0 ** -15)
                tmp = tsp.tile([P, 512], F32, tag="tmp")
                nc.vector.scalar_tensor_tensor(
                    out=tmp, in0=t_sb, scalar=1.0, in1=g_ps,
                    op0=ALU.add, op1=ALU.mult)
                gh_bf = gbp.tile([P, 512], BF16, tag="ghbf")
                nc.vector.scalar_tensor_tensor(
                    out=gh_bf, in0=tmp, scalar=2.0 ** -25, in1=h_ps,
                    op0=ALU.mult, op1=ALU.mult)
                # 2-term split: gh8 = fp8(gh_bf) on Pool; ghr8 = gh_bf - gh8
                nc.gpsimd.tensor_scalar(out=ghq8[:, ht, :], in0=gh_bf,
                                        scalar1=1.0, scalar2=None,
                                        op0=ALU.mult)
                nc.gpsimd.tensor_sub(ghqr8[:, ht, :], gh_bf, ghq8[:, ht, :])

        def ffn_out(ns, ghq8, ghqr8, fpp, wop, finp):
            """out-projection 3-pass + bias + evac for one slice."""
            qsl = slice(ns * 512, (ns + 1) * 512)
            wor_ = d["wob8"].rearrange("(ht p) f -> p ht f", p=P)
            worr_ = d["wobr8"].rearrange("(ht p) f -> p ht f", p=P)
            for fo in range(NDT):
                wob = wop.tile([P, NHT, P], FP8, tag="wob")
                nc.scalar.dma_start(out=wob, in_=wor_[:, :, fo * P:(fo + 1) * P])
                wobr = wop.tile([P, NHT, P], FP8, tag="wobr")
                nc.gpsimd.dma_start(out=wobr, in_=worr_[:, :, fo * P:(fo + 1) * P])
                fp = fpp.tile([P, 512], F32, tag="fp")
                for hp in range(NHT // 2):
                    s2 = slice(2 * hp, 2 * hp + 2)
                    nc.tensor.matmul(fp, wob[:, s2, :], ghq8[:, s2, :],
                                     start=(hp == 0), stop=False, perf_mode=DR)
                for hp in range(NHT // 2):
                    s2 = slice(2 * hp, 2 * hp + 2)
                    nc.tensor.matmul(fp, wob[:, s2, :], ghqr8[:, s2, :],
                                     start=False, stop=False, perf_mode=DR)
                for hp in range(NHT // 2):
                    s2 = slice(2 * hp, 2 * hp + 2)
                    nc.tensor.matmul(fp, wobr[:, s2, :], ghq8[:, s2, :],
                                     start=False, stop=False, perf_mode=DR)
                # + b_out (scaled 2^14) via rank-1 bf16 matmul
                nc.tensor.matmul(fp, bout_sb[:, fo * P:(fo + 1) * P],
                                 ones_row, start=False, stop=True)
                yt = finp.tile([P, 512], F32, tag="yt")
                nc.vector.scalar_tensor_tensor(
                    out=yt, in0=fp, scalar=2.0 ** -14, in1=x1T[:, fo, qsl],
                    op0=ALU.mult, op1=ALU.add)
                nc.gpsimd.dma_start(out=d["yT"][fo * P:(fo + 1) * P, qsl],
                                    in_=yt)

        with ExitStack() as actx:
            ap = actx.enter_context(tc.tile_pool(name="attn", bufs=1))
            vt = ap.tile([P, NKT, NH, HD + 1], FP8, tag="vt")
            kdrp = actx.enter_context(tc.tile_pool(name="kdr", bufs=1, space="DRAM"))
            kdrs = [kdrp.tile([P, L], FP8, tag=f"kdr{i}", name=f"kdr{i}")
                    for i in range(NDT)]
            qdrs = [kdrp.tile([P, LQ], FP8, tag=f"qdr{i}", name=f"qdr{i}")
                    for i in range(NDT)]
            kthp = actx.enter_context(tc.tile_pool(name="kth", bufs=3))
            attnT = ap.tile([P, NDT, 512], FP8, tag="attnT")
            nc.vector.memset(vt[:, :, :, HD:HD + 1], SV / SA)

            # ---- P0: load x8, rmsnorm stats ----
            with ExitStack() as pctx:
                xp = pctx.enter_context(tc.tile_pool(name="xp", bufs=1))
                n1p = pctx.enter_context(tc.tile_pool(name="n1", bufs=3))
                bcp = pctx.enter_context(tc.tile_pool(name="bc1", bufs=2))
                bcP = pctx.enter_context(tc.tile_pool(name="bcP", bufs=1))
                rscp = pctx.enter_context(
                    tc.tile_pool(name="rsc", bufs=1, space="DRAM"))
                ssp = pctx.enter_context(
                    tc.tile_pool(name="ss1", bufs=2, space="PSUM"))
                prp = pctx.enter_context(
                    tc.tile_pool(name="proj", bufs=4, space="PSUM"))

                x8 = xp.tile([P, NDT, L], FP8, tag="x8")
                bck_all = bcP.tile([P, L // 512, 512], F32, tag="bck")
                bcq_all = bcP.tile([P, LQ // 512, 512], F32, tag="bcq")
                for ls in range(L // 512):
                    sl = slice(ls * 512, (ls + 1) * 512)
                    nc.gpsimd.dma_start(out=x8[:, :, sl], in_=x8Tr[:, :, sl])
                rsc = rscp.tile([L], F32, tag="rsc")
                for ls in range(L // 512):
                    sl = slice(ls * 512, (ls + 1) * 512)
                    ss_ps = ssp.tile([1, 512], F32, tag="ss")
                    for dt_ in range(NDT):
                        sq = n1p.tile([P, 512], BF16, tag="sq")
                        nc.gpsimd.tensor_mul(sq, x8[:, dt_, sl], x8[:, dt_, sl])
                        nc.tensor.matmul(ss_ps, ones_col, sq,
                                         start=(dt_ == 0), stop=(dt_ == NDT - 1))
                    lnr = bcp.tile([1, 512], F32, tag="lnr")
                    nc.scalar.activation(lnr, ss_ps, AF.Ln,
                                         bias=eps_t, scale=2.0 ** -18)
                    # rr_k = rr * 2^-10  (K evac, V evac);  rr_q = rr * 2^-9
                    rrk = bcp.tile([1, 512], F32, tag="rrk")
                    nc.scalar.activation(rrk, lnr, AF.Exp, scale=-0.5,
                                         bias=bm10)
                    nc.gpsimd.partition_broadcast(bck_all[:, ls, :], rrk)
                    nc.sync.dma_start(out=rsc[sl], in_=rrk)
                    if ls < LQ // 512:
                        rrq = bcp.tile([1, 512], F32, tag="rrq")
                        nc.scalar.activation(rrq, lnr, AF.Exp, scale=-0.5,
                                             bias=bm9)
                        nc.gpsimd.partition_broadcast(bcq_all[:, ls, :], rrq)
                rcol = bcP.tile([P, NKT], F32, tag="rcol")
                nc.sync.dma_start(out=rcol,
                                  in_=rsc.rearrange("(tt p) -> p tt", p=P))

                # ---- P1: K/Q/V projections (fp8 DR) ----
                with tc.tile_pool(name="wblk", bufs=2) as wp, \
                     tc.tile_pool(name="kmid", bufs=3) as kmp:
                    for (w_d, drl, ntok, bcl) in (
                            (d["wk8"], kdrs, L, bck_all),
                            (d["wq8"], qdrs, LQ, bcq_all)):
                        wr = w_d.rearrange("(dt p) f -> p dt f", p=P)
                        for ft in range(NDT):
                            wblk = wp.tile([P, NDT, P], FP8, tag="wblk")
                            nc.sync.dma_start(
                                out=wblk, in_=wr[:, :, ft * P:(ft + 1) * P])
                            for ks in range(ntok // 512):
                                sl = slice(ks * 512, (ks + 1) * 512)
                                ps = prp.tile([P, 512], F32, tag="pp")
                                for dp in range(NDT // 2):
                                    s2 = slice(2 * dp, 2 * dp + 2)
                                    nc.tensor.matmul(
                                        ps, wblk[:, s2, :], x8[:, s2, sl],
                                        start=(dp == 0),
                                        stop=(dp == NDT // 2 - 1), perf_mode=DR)
                                km = kmp.tile([P, 512], FP8, tag="km")
                                nc.vector.tensor_mul(km, ps, bcl[:, ks, :])
                                nc.sync.dma_start(
                                    out=drl[ft][:, sl], in_=km)
                    # V: token-major psum, ACT evac with per-token scale
                    wvr = d["wv8"].rearrange("(dt p) f -> p dt f", p=P)
                    for hf in range(2):
                        wv = wp.tile([P, NDT, 512], FP8, tag="wblk")
                        nc.sync.dma_start(
                            out=wv, in_=wvr[:, :, hf * 512:(hf + 1) * 512])
                        for tt in range(NKT):
                            ps = prp.tile([P, 512], F32, tag="pp")
                            for dp in range(NDT // 2):
                                s2 = slice(2 * dp, 2 * dp + 2)
                                nc.tensor.matmul(
                                    ps, x8[:, s2, tt * P:(tt + 1) * P],
                                    wv[:, s2, :],
                                    start=(dp == 0),
                                    stop=(dp == NDT // 2 - 1), perf_mode=DR)
                            nc.scalar.activation(
                                vt[:, tt, hf * 8:(hf + 1) * 8, 0:HD],
                                ps.rearrange("p (h e) -> p h e", h=8),
                                AF.Copy, scale=rcol[:, tt:tt + 1])

            # repartitioned K/Q views for DoubleRow scores (streamed per head):
            # within ft: partition p = h2*64 + hi*32 + lo ; head = 2*ft + h2
            kres = [t.rearrange("(h2 hi lo) k -> lo hi h2 k", h2=2, hi=2, lo=32)
                    for t in kdrs]
            qres = [t.rearrange("(h2 hi lo) k -> lo hi h2 k", h2=2, hi=2, lo=32)
                    for t in qdrs]

            # ---- P2: attention per 512-query slice ----
            with ExitStack() as ectx:
                ptp = ectx.enter_context(tc.tile_pool(name="pt", bufs=3))
                smp = ectx.enter_context(tc.tile_pool(name="sm", bufs=2))
                rbp = ectx.enter_context(tc.tile_pool(name="rb", bufs=2))
                xqp = ectx.enter_context(tc.tile_pool(name="xq", bufs=3))
                sqp = ectx.enter_context(tc.tile_pool(name="sq2p", bufs=2))
                wop = ectx.enter_context(tc.tile_pool(name="wo", bufs=2))
                tsp0 = ectx.enter_context(tc.tile_pool(name="ts0", bufs=3))
                gbp0 = ectx.enter_context(tc.tile_pool(name="gb0", bufs=4))
                wfp0 = ectx.enter_context(tc.tile_pool(name="wf0", bufs=2))
                stp = ectx.enter_context(
                    tc.tile_pool(name="st", bufs=2, space="PSUM"))
                accp = ectx.enter_context(
                    tc.tile_pool(name="acc", bufs=2, space="PSUM"))
                gcount = 0
                for ns in range(2):
                    qsl = slice(ns * 512, (ns + 1) * 512)
                    for h in range(NH):
                        dt_ = h // 2
                        r0 = (h % 2) * HD
                        kth = kthp.tile([32, 2, L], FP8, tag="kth")
                        nc.scalar.dma_start(out=kth, in_=kres[h // 2][:, :, h % 2, :])
                        qth = kthp.tile([32, 2, 512], FP8, tag="qth")
                        nc.gpsimd.dma_start(out=qth, in_=qres[h // 2][:, :, h % 2, qsl])
                        acc = accp.tile([HD + 1, 512], F32, tag="acc")
                        for g in range(NKT // 2):
                            st = stp.tile([P, 2, 512], F32, tag="st")
                            for j in range(2):
                                kt = 2 * g + j
                                nc.tensor.matmul(
                                    st[:, j, :],
                                    kth[:, :, kt * P:(kt + 1) * P],
                                    qth,
                                    start=True, stop=True, perf_mode=DR)
                            pt = ptp.tile([P, 2, 512], FP8, tag="pt")
                            if gcount % EXP_DVE_MOD == EXP_DVE_MOD - 1:
                                nc.vector.scalar_tensor_tensor(
                                    out=pt.bitcast(U8), in0=st, scalar=K2B,
                                    in1=bconst, op0=ALU.mult, op1=ALU.add)
                            else:
                                nc.scalar.activation(pt, st, AF.Exp,
                                                     scale=1.0 / (SKQ * SQ2))
                            gcount += 1
                            nc.tensor.matmul(
                                acc, vt[:, 2 * g:2 * g + 2, h, :], pt,
                                start=(g == 0), stop=(g == NKT // 2 - 1),
                                perf_mode=DR)
                        rrow = smp.tile([1, 512], F32, tag="row")
                        nc.vector.reciprocal(rrow, acc[HD:HD + 1, :])
                        rb = rbp.tile([HD, 512], F32, tag="rb")
                        nc.gpsimd.partition_broadcast(rb, rrow)
                        nc.vector.tensor_mul(
                            attnT[r0:r0 + HD, dt_, :], acc[0:HD, :], rb)

                    # Wo projection (fp8 DR) + residual -> x1T
                    wor = d["wo8"].rearrange("(dt p) f -> p dt f", p=P)
                    for ft in range(NDT):
                        wblk = wop.tile([P, NDT, P], FP8, tag="woblk")
                        nc.sync.dma_start(
                            out=wblk, in_=wor[:, :, ft * P:(ft + 1) * P])
                        xq = xqp.tile([P, 512], F32, tag="xq")
                        nc.gpsimd.dma_start(out=xq, in_=xqTr[:, ft, qsl])
                        ps = hpsp.tile([P, 512], F32, tag="h")
                        for dp in range(NDT // 2):
                            s2 = slice(2 * dp, 2 * dp + 2)
                            nc.tensor.matmul(
                                ps, wblk[:, s2, :], attnT[:, s2, :],
                                start=(dp == 0), stop=(dp == NDT // 2 - 1),
                                perf_mode=DR)
                        nc.vector.scalar_tensor_tensor(
                            out=x1T[:, ft, qsl], in0=ps, scalar=2.0 ** -20,
                            in1=xq, op0=ALU.mult, op1=ALU.add)

                    # rmsnorm2 -> x1n fp8 (scale SX2 folded into exp bias)
                    ss2t = gpsp.tile([P, 512], F32, tag="g")
                    ss2 = ss2t[0:1, :]
                    for dt_ in range(NDT):
                        sq2 = sqp.tile([P, 512], BF16, tag="sq2")
                        nc.gpsimd.tensor_mul(sq2, x1T[:, dt_, qsl],
                                             x1T[:, dt_, qsl])
                        nc.tensor.matmul(ss2, ones_col, sq2,
                                         start=(dt_ == 0), stop=(dt_ == NDT - 1))
                    ln2 = smp.tile([1, 512], F32, tag="row")
                    nc.scalar.activation(ln2, ss2, AF.Ln, bias=eps_t,
                                         scale=1.0 / D)
                    rr2 = smp.tile([1, 512], F32, tag="row")
                    nc.scalar.activation(rr2, ln2, AF.Exp, scale=-0.5,
                                         bias=bp4)
                    bc2 = rbp.tile([P, 512], F32, tag="rb2")
                    nc.gpsimd.partition_broadcast(bc2, rr2)
                    x1n = x1n_o.tile([P, NDT, 512], FP8, tag="x1n")
                    for dt_ in range(NDT):
                        nc.gpsimd.tensor_mul(x1n[:, dt_, :], x1T[:, dt_, qsl],
                                             bc2)
                    x1ns.append(x1n)
                    if ns == 0:
                        # FFN slice-0 gate/hidden overlaps slice-1 attention
                        ghq8 = ghp.tile([P, NHT, 512], FP8, tag="gh8")
                        ghqr8 = ghp.tile([P, NHT, 512], FP8, tag="ghr8")
                        ghq_sets.append((ghq8, ghqr8))
                        ffn_gh(0, ghq8, ghqr8, wfp0, tsp0, gbp0,
                               [(gpsp, hpsp)])
            # attention pools closed: PSUM st/acc/mps freed

        # ---- P3: out-proj(0), gate/hidden(1), out-proj(1) ----
        with ExitStack() as fctx:
            fpp = fctx.enter_context(
                tc.tile_pool(name="fpp", bufs=2, space="PSUM"))
            gpsp2 = fctx.enter_context(
                tc.tile_pool(name="gps2", bufs=1, space="PSUM"))
            hpsp2 = fctx.enter_context(
                tc.tile_pool(name="hps2", bufs=1, space="PSUM"))
            tsp = fctx.enter_context(tc.tile_pool(name="tsb", bufs=3))
            gbp = fctx.enter_context(tc.tile_pool(name="gb1", bufs=4))
            wfp = fctx.enter_context(tc.tile_pool(name="wffn", bufs=2))
            wop2 = fctx.enter_context(tc.tile_pool(name="wob2", bufs=2))
            finp = fctx.enter_context(tc.tile_pool(name="fin", bufs=2))

            ghq8, ghqr8 = ghq_sets[0]
            ffn_out(0, ghq8, ghqr8, fpp, wop2, finp)
            ghq8b = ghp.tile([P, NHT, 512], FP8, tag="gh8")
            ghqr8b = ghp.tile([P, NHT, 512], FP8, tag="ghr8")
            ffn_gh(1, ghq8b, ghqr8b, wfp, tsp, gbp,
                   [(gpsp, hpsp), (gpsp2, hpsp2)])
            ffn_out(1, ghq8b, ghqr8b, fpp, wop2, finp)


_NC_CACHE = {}


def kernel(x, W_q, W_k, W_v, W_o, b_o, attn_norm_w, ffn_norm_w,
           W_gate, W_hidden, W_out, b_out):
    x = np.asarray(x, np.float32)
    w1 = np.asarray(attn_norm_w, np.float32)[:, None]
    w2 = np.asarray(ffn_norm_w, np.float32)[:, None]

    def q8(a, sc):
        y = np.ascontiguousarray(np.asarray(a, np.float32)) * sc
        return y.astype(E4)

    def q8pair(a, sc):
        y = np.ascontiguousarray(np.asarray(a, np.float32)) * sc
        a1 = y.astype(E4)
        r = (y - a1.astype(np.float32)).astype(E4)
        return a1, r

    wq8 = q8(np.asarray(W_q, np.float32).T * w1 / np.sqrt(HD), SWQ)
    wk8 = q8(np.asarray(W_k, np.float32).T * w1, SWK)
    wv8 = q8(np.asarray(W_v, np.float32).T * w1, SWV)
    wo8 = q8(np.asarray(W_o, np.float32).T, SWO)
    wg8, wgr8 = q8pair(np.asarray(W_gate, np.float32).T * w2, SWF)
    wh8, whr8 = q8pair(np.asarray(W_hidden, np.float32).T * w2, SWF)
    wob8, wobr8 = q8pair(np.asarray(W_out, np.float32).T, SWF)
    bout_row = np.ascontiguousarray(
        (np.asarray(b_out, np.float32) * (SWF * SGH))[None, :]
    ).astype(ml_dtypes.bfloat16)
    bo = np.asarray(b_o, np.float32)

    if "nc" not in _NC_CACHE:
        _NC_CACHE["nc"] = build_nc()
    nc = _NC_CACHE["nc"]

    in_maps = []
    for c in range(8):
        b, half = c // 2, c % 2
        xb = x[b]
        if half:
            xb = np.concatenate([xb[LQ:], xb[:LQ]], axis=0)
        in_maps.append({
            "x8T": np.ascontiguousarray((xb.T * SX1)).astype(E4),
            "xqT": np.ascontiguousarray(xb[:LQ].T + bo[:, None]),
            "wq8": wq8, "wk8": wk8, "wv8": wv8, "wo8": wo8,
            "wg8": wg8, "wgr8": wgr8, "wh8": wh8, "whr8": whr8,
            "wob8": wob8, "wobr8": wobr8, "bout_row": bout_row,
        })
    res = bass_utils.run_bass_kernel_spmd(nc, in_maps, core_ids=list(range(8)))
    y = np.empty((4, L, D), np.float32)
    for c in range(8):
        b, half = c // 2, c % 2
        y[b, half * LQ:(half + 1) * LQ, :] = res.results[c]["yT"].T
    return y


# revision 17
# speedup vs baseline: 1.5105x; 1.0645x over previous
"""Dense transformer block (RMSNorm+MHA+residual, RMSNorm+SwiGLU+residual)
on 8 trn2 NeuronCores. Sharding: 2 cores per batch element; each core
computes the block output for 1024 of its batch's 2048 tokens, redundantly
computing K/V for the full sequence (keys are permutation invariant; each
core's x puts its own 1024 query tokens first). No inter-core communication.

fp8 (e4m3) DoubleRow design: every large matmul runs fp8 with
MatmulPerfMode.DoubleRow (0.5 cycles/row, 256-deep contraction pairs).
Numerics validated in numpy + on HW (max abs err ~0.07 vs gate 0.109):
- attention path entirely 1-term fp8 (x, wq/wk/wv/wo, k/q/v, probs, attn)
  with power-of-2 scales; rmsnorm scales folded into PSUM evacuations and
  host-prefolded weights.
- FFN: 1-term fp8 weights x 2-term (flat-scale residual) fp8 x1n for
  gate/hidden; out-proj 3-pass (w8*gh8 + w8*ghr8 + wr8*gh8) with the
  2-term gh split computed on Pool (cast + subtract from a bf16 master).
- softmax exp split between ACT (table exp -> fp8) and DVE (Schraudolph
  uint8 bit-trick via tensor_scalar, bitcast into the same fp8 tile);
  softmax normalization cancels the bit-trick's systematic error.
- scores use a repartitioned K/Q layout [32(lo), 2(h2), 2(hi), tokens]
  per 2-head group, produced via a DRAM round-trip, so DoubleRow can pair
  the two 32-feature halves of each 64-wide head.
- silu via tanh identity keeps ACT on the exp-compatible table set;
  FFN(slice0) gate/hidden matmuls overlap the slice-1 attention window.
- DMA discipline: host-side weight layouts give >=1KB contiguous runs and
  one DMA per tile group (~150 DMAs total); HWDGE triggers on the
  otherwise-idle SP engine (each holds SEQ+HWDGE ~700ns), bulk x8/y on
  gpsimd SWDGE.
"""
import sys
from contextlib import ExitStack

import numpy as np

sys.path.insert(0, "/opt/trn_rl_repo")

import ml_dtypes  # noqa: E402
import concourse.bass as bass  # noqa: E402
from concourse import bacc  # noqa: E402
import concourse.tile as tile  # noqa: E402
from concourse import mybir  # noqa: E402
from concourse import bass_utils  # noqa: E402

P = 128
D = 1024          # d_model
L = 2048          # full seq per core (keys)
LQ = 1024         # query tokens per core
NH = 16
HD = 64
HID = 4096
EPS = 1e-6
NDT = D // P      # 8 feature tiles
NKT = L // P      # 16 key tiles
NHT = HID // P    # 32 hidden tiles
LN2 = float(np.log(2.0))

# power-of-2 fp8 scales (validated in acc_sim.py)
SX1 = 16.0        # x8 = fp8(x * SX1)
SWQ, SWK, SWV, SWO = 8192.0, 2048.0, 2048.0, 1024.0
SKQ, SQ2, SV, SA = 32.0, 256.0, 32.0, 1024.0
SX2, SWF, SGH = 16.0, 1024.0, 16.0
# Schraudolph exp on DVE: uint8 bits = st*K2B + BCONST, bitcast to e4m3
K2B = float(8.0 * np.log2(np.e) / (SKQ * SQ2))   # st = 8192 * s_true
BCONST = 55.5                                     # 7*8 + c_adj(-0.5)
EXP_DVE_MOD = 4   # every 4th exp group goes to DVE

F32 = mybir.dt.float32
BF16 = mybir.dt.bfloat16
FP8 = mybir.dt.float8e4
U8 = mybir.dt.uint8
AF = mybir.ActivationFunctionType
ALU = mybir.AluOpType
DR = mybir.MatmulPerfMode.DoubleRow
E4 = ml_dtypes.float8_e4m3

SIM_TIME_NS = None


def build_nc():
    global SIM_TIME_NS
    nc = bacc.Bacc(None, target_bir_lowering=False)
    d = {}
    d["x8T"] = nc.dram_tensor("x8T", [D, L], FP8, kind="ExternalInput")
    d["xqT"] = nc.dram_tensor("xqT", [D, LQ], F32, kind="ExternalInput")
    d["wkq8"] = nc.dram_tensor("wkq8", [P, NDT, 2, NDT, P], FP8,
                               kind="ExternalInput")
    d["wv8"] = nc.dram_tensor("wv8", [D, D], FP8, kind="ExternalInput")
    d["wo8h"] = nc.dram_tensor("wo8h", [P, NDT, NDT, P], FP8,
                               kind="ExternalInput")
    d["wgh8"] = nc.dram_tensor("wgh8", [P, NHT, 2, NDT, P], FP8,
                               kind="ExternalInput")
    d["wobc8"] = nc.dram_tensor("wobc8", [P, NDT, 2, NHT, P], FP8,
                                kind="ExternalInput")
    d["bout_row"] = nc.dram_tensor("bout_row", [1, D], BF16,
                                   kind="ExternalInput")
    d["yT"] = nc.dram_tensor("yT", [D, LQ], F32, kind="ExternalOutput")

    with tile.TileContext(nc) as tc:
        _body(tc, nc, d)
        _, snap = tc.schedule_and_allocate()
        SIM_TIME_NS = snap.time
    nc.compile()
    return nc


def _body(tc, nc, d):
    x8Tr = d["x8T"].rearrange("(dt p) l -> p dt l", p=P)
    xqTr = d["xqT"].rearrange("(dt p) l -> p dt l", p=P)
    yTr = d["yT"].rearrange("(dt p) l -> p dt l", p=P)

    with ExitStack() as pp_ctx:
        pp = pp_ctx.enter_context(tc.tile_pool(name="persist", bufs=1))
        eps_t = pp.tile([1, 1], F32, tag="eps")
        bm10 = pp.tile([1, 1], F32, tag="bm10")
        bm9 = pp.tile([1, 1], F32, tag="bm9")
        bp4 = pp.tile([1, 1], F32, tag="bp4")
        ones_col = pp.tile([P, 1], BF16, tag="ones")
        ones_row = pp.tile([1, 512], BF16, tag="onesr")
        bout_sb = pp.tile([1, D], BF16, tag="bout")
        x1T = pp.tile([P, NDT, LQ], F32, tag="x1T")
        x1n_o = pp_ctx.enter_context(tc.tile_pool(name="x1n", bufs=2))
        nc.vector.memset(eps_t, EPS)
        nc.vector.memset(bm10, -10.0 * LN2)
        nc.vector.memset(bm9, -9.0 * LN2)
        nc.vector.memset(bp4, 4.0 * LN2)
        nc.vector.memset(ones_col, 1.0)
        nc.vector.memset(ones_row, 1.0)
        nc.sync.dma_start(out=bout_sb, in_=d["bout_row"][:, :])
        x1ns = []
        ghq_sets = []

        # FFN PSUM pools allocated early (shared across overlap window)
        gpsp = pp_ctx.enter_context(tc.tile_pool(name="gps", bufs=1, space="PSUM"))
        hpsp = pp_ctx.enter_context(tc.tile_pool(name="hps", bufs=1, space="PSUM"))

        def ffn_gh(ns, ghq8, ghqr8, wfp, tsp, gbp, pools):
            """gate/hidden (2-pass over x1n8/x1nr8) + silu chain -> gh tiles."""
            x1n8, x1nr8 = x1ns[ns]
            for jj in range(NHT // 2):
                wgh2 = wfp.tile([P, 2, 2, NDT, P], FP8, tag="wgh2")
                nc.sync.dma_start(out=wgh2,
                                  in_=d["wgh8"][:, 2 * jj:2 * jj + 2, :, :, :])
                for j in range(2):
                    ht = 2 * jj + j
                    gpool, hpool = pools[ht % len(pools)]
                    g_ps = gpool.tile([P, 512], F32, tag="g")
                    for dp in range(NDT // 2):
                        s2 = slice(2 * dp, 2 * dp + 2)
                        nc.tensor.matmul(g_ps, wgh2[:, j, 0, s2, :],
                                         x1n8[:, s2, :],
                                         start=(dp == 0), stop=False,
                                         perf_mode=DR)
                    for dp in range(NDT // 2):
                        s2 = slice(2 * dp, 2 * dp + 2)
                        nc.tensor.matmul(g_ps, wgh2[:, j, 0, s2, :],
                                         x1nr8[:, s2, :],
                                         start=False, stop=(dp == NDT // 2 - 1),
                                         perf_mode=DR)
                    h_ps = hpool.tile([P, 512], F32, tag="h")
                    for dp in range(NDT // 2):
                        s2 = slice(2 * dp, 2 * dp + 2)
                        nc.tensor.matmul(h_ps, wgh2[:, j, 1, s2, :],
                                         x1n8[:, s2, :],
                                         start=(dp == 0), stop=False,
                                         perf_mode=DR)
                    for dp in range(NDT // 2):
                        s2 = slice(2 * dp, 2 * dp + 2)
                        nc.tensor.matmul(h_ps, wgh2[:, j, 1, s2, :],
                                         x1nr8[:, s2, :],
                                         start=False, stop=(dp == NDT // 2 - 1),
                                         perf_mode=DR)
                    # silu(g)*h via tanh: t = tanh(G/2); gh = 0.5*G*(1+t)*H
                    t_sb = tsp.tile([P, 512], F32, tag="tanh")
                    nc.scalar.activation(t_sb, g_ps, AF.Tanh, scale=2.0 ** -15)
                    tmp = tsp.tile([P, 512], F32, tag="tmp")
                    nc.vector.scalar_tensor_tensor(
                        out=tmp, in0=t_sb, scalar=1.0, in1=g_ps,
                        op0=ALU.add, op1=ALU.mult)
                    gh_bf = gbp.tile([P, 512], BF16, tag="ghbf")
                    nc.vector.scalar_tensor_tensor(
                        out=gh_bf, in0=tmp, scalar=2.0 ** -25, in1=h_ps,
                        op0=ALU.mult, op1=ALU.mult)
                    nc.gpsimd.tensor_scalar(out=ghq8[:, ht, :], in0=gh_bf,
                                            scalar1=1.0, scalar2=None,
                                            op0=ALU.mult)
                    nc.gpsimd.tensor_sub(ghqr8[:, ht, :], gh_bf,
                                         ghq8[:, ht, :])

        def ffn_out(fpp, wop, finp):
            """out-projection 3-pass, fo-outer over both slices."""
            ybufs = [finp.tile([P, NDT, 512], F32, tag="yb0", name="yb0"),
                     finp.tile([P, NDT, 512], F32, tag="yb1", name="yb1")]
            for fo in range(NDT):
                wobc = wop.tile([P, 2, NHT, P], FP8, tag="wobc")
                nc.sync.dma_start(out=wobc, in_=d["wobc8"][:, fo, :, :, :])
                for ns in range(2):
                    ghq8, ghqr8 = ghq_sets[ns]
                    qsl = slice(ns * 512, (ns + 1) * 512)
                    fp = fpp.tile([P, 512], F32, tag="fp")
                    for hp in range(NHT // 2):
                        s2 = slice(2 * hp, 2 * hp + 2)
                        nc.tensor.matmul(fp, wobc[:, 0, s2, :], ghq8[:, s2, :],
                                         start=(hp == 0), stop=False,
                                         perf_mode=DR)
                    for hp in range(NHT // 2):
                        s2 = slice(2 * hp, 2 * hp + 2)
                        nc.tensor.matmul(fp, wobc[:, 0, s2, :], ghqr8[:, s2, :],
                                         start=False, stop=False, perf_mode=DR)
                    for hp in range(NHT // 2):
                        s2 = slice(2 * hp, 2 * hp + 2)
                        nc.tensor.matmul(fp, wobc[:, 1, s2, :], ghq8[:, s2, :],
                                         start=False, stop=False, perf_mode=DR)
                    # + b_out (scaled 2^14) via rank-1 bf16 matmul
                    nc.tensor.matmul(fp, bout_sb[:, fo * P:(fo + 1) * P],
                                     ones_row, start=False, stop=True)
                    nc.vector.scalar_tensor_tensor(
                        out=ybufs[ns][:, fo, :], in0=fp, scalar=2.0 ** -14,
                        in1=x1T[:, fo, qsl], op0=ALU.mult, op1=ALU.add)
            for ns in range(2):
                qsl = slice(ns * 512, (ns + 1) * 512)
                nc.gpsimd.dma_start(out=yTr[:, :, qsl], in_=ybufs[ns])

        ghp = pp_ctx.enter_context(tc.tile_pool(name="ghq", bufs=1))
        with ExitStack() as actx:
            ap = actx.enter_context(tc.tile_pool(name="attn", bufs=1))
            vt = ap.tile([P, NKT, NH, HD + 1], FP8, tag="vt")
            attnT = ap.tile([P, NDT, 512], FP8, tag="attnT")
            wo_all = ap.tile([P, NDT, NDT, P], FP8, tag="wo_all")
            kdrp = actx.enter_context(
                tc.tile_pool(name="kdr", bufs=1, space="DRAM"))
            kdrs = [kdrp.tile([P, L], FP8, tag=f"kdr{i}", name=f"kdr{i}")
                    for i in range(NDT)]
            qdrs = [kdrp.tile([P, LQ], FP8, tag=f"qdr{i}", name=f"qdr{i}")
                    for i in range(NDT)]
            nc.vector.memset(vt[:, :, :, HD:HD + 1], SV / SA)
            nc.sync.dma_start(out=wo_all, in_=d["wo8h"][:, :, :, :])

            # ---- P0: load x8, rmsnorm stats ----
            with ExitStack() as pctx:
                xp = pctx.enter_context(tc.tile_pool(name="xp", bufs=1))
                n1p = pctx.enter_context(tc.tile_pool(name="n1", bufs=3))
                bcp = pctx.enter_context(tc.tile_pool(name="bc1", bufs=2))
                bcP = pctx.enter_context(tc.tile_pool(name="bcP", bufs=1))
                rscp = pctx.enter_context(
                    tc.tile_pool(name="rsc", bufs=1, space="DRAM"))
                ssp = pctx.enter_context(
                    tc.tile_pool(name="ss1", bufs=2, space="PSUM"))
                prp = pctx.enter_context(
                    tc.tile_pool(name="proj", bufs=4, space="PSUM"))

                x8 = xp.tile([P, NDT, L], FP8, tag="x8")
                bck_all = bcP.tile([P, L // 512, 512], F32, tag="bck")
                bcq_all = bcP.tile([P, LQ // 512, 512], F32, tag="bcq")
                for ls in range(L // 512):
                    sl = slice(ls * 512, (ls + 1) * 512)
                    nc.gpsimd.dma_start(out=x8[:, :, sl], in_=x8Tr[:, :, sl])
                rsc = rscp.tile([L], F32, tag="rsc")
                for ls in range(L // 512):
                    sl = slice(ls * 512, (ls + 1) * 512)
                    ss_ps = ssp.tile([1, 512], F32, tag="ss")
                    for dt_ in range(NDT):
                        sq = n1p.tile([P, 512], BF16, tag="sq")
                        nc.gpsimd.tensor_mul(sq, x8[:, dt_, sl], x8[:, dt_, sl])
                        nc.tensor.matmul(ss_ps, ones_col, sq,
                                         start=(dt_ == 0), stop=(dt_ == NDT - 1))
                    lnr = bcp.tile([1, 512], F32, tag="lnr")
                    nc.scalar.activation(lnr, ss_ps, AF.Ln,
                                         bias=eps_t, scale=2.0 ** -18)
                    # rr_k = rr * 2^-10  (K evac, V evac);  rr_q = rr * 2^-9
                    rrk = bcp.tile([1, 512], F32, tag="rrk")
                    nc.scalar.activation(rrk, lnr, AF.Exp, scale=-0.5,
                                         bias=bm10)
                    nc.gpsimd.partition_broadcast(bck_all[:, ls, :], rrk)
                    nc.sync.dma_start(out=rsc[sl], in_=rrk)
                    if ls < LQ // 512:
                        rrq = bcp.tile([1, 512], F32, tag="rrq")
                        nc.scalar.activation(rrq, lnr, AF.Exp, scale=-0.5,
                                             bias=bm9)
                        nc.gpsimd.partition_broadcast(bcq_all[:, ls, :], rrq)
                rcol = bcP.tile([P, NKT], F32, tag="rcol")
                nc.sync.dma_start(out=rcol,
                                  in_=rsc.rearrange("(tt p) -> p tt", p=P))

                # ---- P1: K/Q/V projections (fp8 DR), DRAM round-trip ----
                with tc.tile_pool(name="wblk", bufs=2) as wp, \
                     tc.tile_pool(name="kmid", bufs=2) as kmp:
                    for ft in range(NDT):
                        wkq = wp.tile([P, 2, NDT, P], FP8, tag="wkq")
                        nc.sync.dma_start(out=wkq, in_=d["wkq8"][:, ft, :, :, :])
                        kmK = kmp.tile([P, L], FP8, tag="kmK")
                        for ks in range(L // 512):
                            sl = slice(ks * 512, (ks + 1) * 512)
                            ps = prp.tile([P, 512], F32, tag="pp")
                            for dp in range(NDT // 2):
                                s2 = slice(2 * dp, 2 * dp + 2)
                                nc.tensor.matmul(
                                    ps, wkq[:, 0, s2, :], x8[:, s2, sl],
                                    start=(dp == 0),
                                    stop=(dp == NDT // 2 - 1), perf_mode=DR)
                            nc.vector.tensor_mul(kmK[:, sl], ps,
                                                 bck_all[:, ks, :])
                        nc.sync.dma_start(out=kdrs[ft][:, :], in_=kmK)
                        kmQ = kmp.tile([P, LQ], FP8, tag="kmQ")
                        for ks in range(LQ // 512):
                            sl = slice(ks * 512, (ks + 1) * 512)
                            ps = prp.tile([P, 512], F32, tag="pp")
                            for dp in range(NDT // 2):
                                s2 = slice(2 * dp, 2 * dp + 2)
                                nc.tensor.matmul(
                                    ps, wkq[:, 1, s2, :], x8[:, s2, sl],
                                    start=(dp == 0),
                                    stop=(dp == NDT // 2 - 1), perf_mode=DR)
                            nc.vector.tensor_mul(kmQ[:, sl], ps,
                                                 bcq_all[:, ks, :])
                        nc.sync.dma_start(out=qdrs[ft][:, :], in_=kmQ)
                    # V: token-major psum, ACT evac with per-token scale
                    wvr = d["wv8"].rearrange("(dt p) f -> p dt f", p=P)
                    for hf in range(2):
                        wv = wp.tile([P, NDT, 512], FP8, tag="wv")
                        nc.sync.dma_start(
                            out=wv, in_=wvr[:, :, hf * 512:(hf + 1) * 512])
                        for tt in range(NKT):
                            ps = prp.tile([P, 512], F32, tag="pp")
                            for dp in range(NDT // 2):
                                s2 = slice(2 * dp, 2 * dp + 2)
                                nc.tensor.matmul(
                                    ps, x8[:, s2, tt * P:(tt + 1) * P],
                                    wv[:, s2, :],
                                    start=(dp == 0),
                                    stop=(dp == NDT // 2 - 1), perf_mode=DR)
                            nc.scalar.activation(
                                vt[:, tt, hf * 8:(hf + 1) * 8, 0:HD],
                                ps.rearrange("p (h e) -> p h e", h=8),
                                AF.Copy, scale=rcol[:, tt:tt + 1])

            # repartitioned K/Q views (per 2-head ft group):
            # within ft: partition p = h2*64 + hi*32 + lo ; head = 2*ft + h2
            kres = [t.rearrange("(h2 hi lo) k -> lo h2 hi k", h2=2, hi=2, lo=32)
                    for t in kdrs]
            qres = [t.rearrange("(h2 hi lo) k -> lo h2 hi k", h2=2, hi=2, lo=32)
                    for t in qdrs]

            # ---- P2: attention per 512-query slice ----
            with ExitStack() as ectx:
                kthp = ectx.enter_context(tc.tile_pool(name="kth", bufs=2))
                ptp = ectx.enter_context(tc.tile_pool(name="pt", bufs=3))
                smp = ectx.enter_context(tc.tile_pool(name="sm", bufs=2))
                rbp = ectx.enter_context(tc.tile_pool(name="rb", bufs=2))
                xqp = ectx.enter_context(tc.tile_pool(name="xq", bufs=1))
                sqp = ectx.enter_context(tc.tile_pool(name="sq2p", bufs=2))
                tsp0 = ectx.enter_context(tc.tile_pool(name="ts0", bufs=3))
                gbp0 = ectx.enter_context(tc.tile_pool(name="gb0", bufs=4))
                wfp0 = ectx.enter_context(tc.tile_pool(name="wf0", bufs=2))
                x1bp = ectx.enter_context(tc.tile_pool(name="x1b", bufs=3))
                stp = ectx.enter_context(
                    tc.tile_pool(name="st", bufs=2, space="PSUM"))
                accp = ectx.enter_context(
                    tc.tile_pool(name="acc", bufs=2, space="PSUM"))
                gcount = 0
                for ns in range(2):
                    qsl = slice(ns * 512, (ns + 1) * 512)
                    xqs = xqp.tile([P, NDT, 512], F32, tag="xqs")
                    nc.sync.dma_start(out=xqs, in_=xqTr[:, :, qsl])
                    for ft in range(NDT):
                        kth = kthp.tile([32, 2, 2, L], FP8, tag="kth")
                        nc.sync.dma_start(out=kth, in_=kres[ft])
                        qth = kthp.tile([32, 2, 2, 512], FP8, tag="qth")
                        nc.sync.dma_start(out=qth, in_=qres[ft][:, :, :, qsl])
                        for h2 in range(2):
                            h = 2 * ft + h2
                            r0 = h2 * HD
                            acc = accp.tile([HD + 1, 512], F32, tag="acc")
                            for g in range(NKT // 2):
                                st = stp.tile([P, 2, 512], F32, tag="st")
                                for j in range(2):
                                    kt = 2 * g + j
                                    nc.tensor.matmul(
                                        st[:, j, :],
                                        kth[:, h2, :, kt * P:(kt + 1) * P],
                                        qth[:, h2, :, :],
                                        start=True, stop=True, perf_mode=DR)
                                pt = ptp.tile([P, 2, 512], FP8, tag="pt")
                                if gcount % EXP_DVE_MOD == EXP_DVE_MOD - 1:
                                    nc.vector.tensor_scalar(
                                        out=pt.bitcast(U8), in0=st,
                                        scalar1=K2B, scalar2=BCONST,
                                        op0=ALU.mult, op1=ALU.add)
                                else:
                                    nc.scalar.activation(
                                        pt, st, AF.Exp, scale=1.0 / (SKQ * SQ2))
                                gcount += 1
                                nc.tensor.matmul(
                                    acc, vt[:, 2 * g:2 * g + 2, h, :], pt,
                                    start=(g == 0), stop=(g == NKT // 2 - 1),
                                    perf_mode=DR)
                            rrow = smp.tile([1, 512], F32, tag="row")
                            nc.vector.reciprocal(rrow, acc[HD:HD + 1, :])
                            rb = rbp.tile([HD, 512], F32, tag="rb")
                            nc.gpsimd.partition_broadcast(rb, rrow)
                            nc.vector.tensor_mul(
                                attnT[r0:r0 + HD, ft, :], acc[0:HD, :], rb)

                    # Wo projection (fp8 DR) + residual -> x1T
                    for ft in range(NDT):
                        ps = hpsp.tile([P, 512], F32, tag="h")
                        for dp in range(NDT // 2):
                            s2 = slice(2 * dp, 2 * dp + 2)
                            nc.tensor.matmul(
                                ps, wo_all[:, ft, s2, :], attnT[:, s2, :],
                                start=(dp == 0), stop=(dp == NDT // 2 - 1),
                                perf_mode=DR)
                        nc.vector.scalar_tensor_tensor(
                            out=x1T[:, ft, qsl], in0=ps, scalar=2.0 ** -20,
                            in1=xqs[:, ft, :], op0=ALU.mult, op1=ALU.add)

                    # rmsnorm2 -> x1n8 + x1nr8 (2-term fp8, Pool chain)
                    ss2t = gpsp.tile([P, 512], F32, tag="g")
                    ss2 = ss2t[0:1, :]
                    for dt_ in range(NDT):
                        sq2 = sqp.tile([P, 512], BF16, tag="sq2")
                        nc.gpsimd.tensor_mul(sq2, x1T[:, dt_, qsl],
                                             x1T[:, dt_, qsl])
                        nc.tensor.matmul(ss2, ones_col, sq2,
                                         start=(dt_ == 0), stop=(dt_ == NDT - 1))
                    ln2 = smp.tile([1, 512], F32, tag="row")
                    nc.scalar.activation(ln2, ss2, AF.Ln, bias=eps_t,
                                         scale=1.0 / D)
                    rr2 = smp.tile([1, 512], F32, tag="row")
                    nc.scalar.activation(rr2, ln2, AF.Exp, scale=-0.5,
                                         bias=bp4)
                    bc2 = rbp.tile([P, 512], F32, tag="rb2")
                    nc.gpsimd.partition_broadcast(bc2, rr2)
                    x1n8 = x1n_o.tile([P, NDT, 512], FP8, tag="x1n8")
                    x1nr8 = x1n_o.tile([P, NDT, 512], FP8, tag="x1nr8")
                    for dt_ in range(NDT):
                        x1b = x1bp.tile([P, 512], BF16, tag="x1b")
                        nc.gpsimd.tensor_mul(x1b, x1T[:, dt_, qsl], bc2)
                        nc.gpsimd.tensor_scalar(out=x1n8[:, dt_, :], in0=x1b,
                                                scalar1=1.0, scalar2=None,
                                                op0=ALU.mult)
                        nc.gpsimd.tensor_sub(x1nr8[:, dt_, :], x1b,
                                             x1n8[:, dt_, :])
                    x1ns.append((x1n8, x1nr8))
                    if ns == 0:
                        ghq8 = ghp.tile([P, NHT, 512], FP8, tag="gh8")
                        ghqr8 = ghp.tile([P, NHT, 512], FP8, tag="ghr8")
                        ghq_sets.append((ghq8, ghqr8))
                        ffn_gh(0, ghq8, ghqr8, wfp0, tsp0, gbp0,
                               [(gpsp, hpsp)])
            # attention pools closed: PSUM st/acc freed

        # ---- P3: gate/hidden(1), then fo-outer out-proj over both slices ----
        with ExitStack() as fctx:
            fpp = fctx.enter_context(
                tc.tile_pool(name="fpp", bufs=2, space="PSUM"))
            gpsp2 = fctx.enter_context(
                tc.tile_pool(name="gps2", bufs=1, space="PSUM"))
            hpsp2 = fctx.enter_context(
                tc.tile_pool(name="hps2", bufs=1, space="PSUM"))
            tsp = fctx.enter_context(tc.tile_pool(name="tsb", bufs=3))
            gbp = fctx.enter_context(tc.tile_pool(name="gb1", bufs=4))
            wfp = fctx.enter_context(tc.tile_pool(name="wffn", bufs=2))
            wop2 = fctx.enter_context(tc.tile_pool(name="wob2", bufs=2))
            finp = fctx.enter_context(tc.tile_pool(name="fin", bufs=1))

            ghpb = fctx.enter_context(tc.tile_pool(name="ghqb", bufs=1))
            ghq8b = ghpb.tile([P, NHT, 512], FP8, tag="gh8b")
            ghqr8b = ghpb.tile([P, NHT, 512], FP8, tag="ghr8b")
            ghq_sets.append((ghq8b, ghqr8b))
            ffn_gh(1, ghq8b, ghqr8b, wfp, tsp, gbp,
                   [(gpsp, hpsp), (gpsp2, hpsp2)])
            ffn_out(fpp, wop2, finp)


_NC_CACHE = {}


def kernel(x, W_q, W_k, W_v, W_o, b_o, attn_norm_w, ffn_norm_w,
           W_gate, W_hidden, W_out, b_out):
    x = np.asarray(x, np.float32)
    w1 = np.asarray(attn_norm_w, np.float32)[:, None]
    w2 = np.asarray(ffn_norm_w, np.float32)[:, None]

    def q8(a, sc):
        return (np.ascontiguousarray(np.asarray(a, np.float32)) * sc).astype(E4)

    def blk4(a):
        """[D, F] -> [P(p), F//128(ft), D//128(dt), 128] weight layout."""
        dd, ff = a.shape
        return np.ascontiguousarray(
            a.reshape(dd // P, P, ff // P, P).transpose(1, 2, 0, 3))

    wq8 = blk4(q8(np.asarray(W_q, np.float32).T * w1 / np.sqrt(HD), SWQ))
    wk8 = blk4(q8(np.asarray(W_k, np.float32).T * w1, SWK))
    wkq8 = np.ascontiguousarray(np.stack([wk8, wq8], axis=2))
    wv8 = q8(np.asarray(W_v, np.float32).T * w1, SWV)
    wo8h = blk4(q8(np.asarray(W_o, np.float32).T, SWO))
    wg8 = blk4(q8(np.asarray(W_gate, np.float32).T * w2, SWF))
    wh8 = blk4(q8(np.asarray(W_hidden, np.float32).T * w2, SWF))
    wgh8 = np.ascontiguousarray(np.stack([wg8, wh8], axis=2))
    wobf = np.ascontiguousarray(np.asarray(W_out, np.float32).T) * SWF
    wob8 = wobf.astype(E4)
    wobr8 = (wobf - wob8.astype(np.float32)).astype(E4)
    wobc8 = np.ascontiguousarray(
        np.stack([blk4(wob8), blk4(wobr8)], axis=2))
    bout_row = np.ascontiguousarray(
        (np.asarray(b_out, np.float32) * (SWF * SGH))[None, :]
    ).astype(ml_dtypes.bfloat16)
    bo = np.asarray(b_o, np.float32)

    if "nc" not in _NC_CACHE:
        _NC_CACHE["nc"] = build_nc()
    nc = _NC_CACHE["nc"]

    in_maps = []
    for c in range(8):
        b, half = c // 2, c % 2
        xb = x[b]
        if half:
            xb = np.concatenate([xb[LQ:], xb[:LQ]], axis=0)
        in_maps.append({
            "x8T": np.ascontiguousarray((xb.T * SX1)).astype(E4),
            "xqT": np.ascontiguousarray(xb[:LQ].T + bo[:, None]),
            "wkq8": wkq8, "wv8": wv8, "wo8h": wo8h,
            "wgh8": wgh8, "wobc8": wobc8, "bout_row": bout_row,
        })
    res = bass_utils.run_bass_kernel_spmd(nc, in_maps, core_ids=list(range(8)))
    y = np.empty((4, L, D), np.float32)
    for c in range(8):
        b, half = c // 2, c % 2
        y[b, half * LQ:(half + 1) * LQ, :] = res.results[c]["yT"].T
    return y


# revision 24
# speedup vs baseline: 1.6267x; 1.0769x over previous
"""Dense transformer block (RMSNorm+MHA+residual, RMSNorm+SwiGLU+residual)
on 8 trn2 NeuronCores. Sharding: 2 cores per batch element; each core
computes the block output for 1024 of its batch's 2048 tokens, redundantly
computing K/V for the full sequence (keys are permutation invariant; each
core's x puts its own 1024 query tokens first). No inter-core communication.

fp8 (e4m3) DoubleRow design: every large matmul runs fp8 with
MatmulPerfMode.DoubleRow (0.5 cycles/row, 256-deep contraction pairs).
Numerics validated in numpy + on HW (max abs err ~0.07 vs gate 0.109):
- attention path entirely 1-term fp8 (x, wq/wk/wv/wo, k/q/v, probs, attn)
  with power-of-2 scales; rmsnorm scales folded into PSUM evacuations and
  host-prefolded weights.
- FFN: 1-term fp8 weights x 2-term (flat-scale residual) fp8 x1n for
  gate/hidden; out-proj 3-pass (w8*gh8 + w8*ghr8 + wr8*gh8) with the
  2-term gh split computed on Pool (cast + subtract from a bf16 master).
- softmax exp split between ACT (table exp -> fp8) and DVE (Schraudolph
  uint8 bit-trick via tensor_scalar, bitcast into the same fp8 tile);
  softmax normalization cancels the bit-trick's systematic error.
- scores use a repartitioned K/Q layout [32(lo), 2(h2), 2(hi), tokens]
  per 2-head group, produced via a DRAM round-trip, so DoubleRow can pair
  the two 32-feature halves of each 64-wide head.
- silu via tanh identity keeps ACT on the exp-compatible table set;
  FFN(slice0) gate/hidden matmuls overlap the slice-1 attention window.
- DMA discipline: host-side weight layouts give >=1KB contiguous runs and
  one DMA per tile group (~150 DMAs total); HWDGE triggers on the
  otherwise-idle SP engine (each holds SEQ+HWDGE ~700ns), bulk x8/y on
  gpsimd SWDGE.
"""
import sys
from contextlib import ExitStack

import numpy as np

sys.path.insert(0, "/opt/trn_rl_repo")

import ml_dtypes  # noqa: E402
import concourse.bass as bass  # noqa: E402
from concourse import bacc  # noqa: E402
import concourse.tile as tile  # noqa: E402
from concourse import mybir  # noqa: E402
from concourse import bass_utils  # noqa: E402

P = 128
D = 1024          # d_model
L = 2048          # full seq per core (keys)
LQ = 1024         # query tokens per core
NH = 16
HD = 64
HID = 4096
EPS = 1e-6
NDT = D // P      # 8 feature tiles
NKT = L // P      # 16 key tiles
NHT = HID // P    # 32 hidden tiles
LN2 = float(np.log(2.0))

# power-of-2 fp8 scales (validated in acc_sim.py)
SX1 = 16.0        # x8 = fp8(x * SX1)
SWQ, SWK, SWV, SWO = 8192.0, 2048.0, 2048.0, 1024.0
SKQ, SQ2, SV, SA = 32.0, 256.0, 32.0, 1024.0
SX2, SWF, SGH = 16.0, 1024.0, 16.0
# Schraudolph exp on DVE: uint8 bits = st*K2B + BCONST, bitcast to e4m3
K2B = float(8.0 * np.log2(np.e) / (SKQ * SQ2))   # st = 8192 * s_true
BCONST = 55.5                                     # 7*8 + c_adj(-0.5)
EXP_DVE_MOD = 4   # every 4th exp group goes to DVE

F32 = mybir.dt.float32
BF16 = mybir.dt.bfloat16
FP8 = mybir.dt.float8e4
U8 = mybir.dt.uint8
AF = mybir.ActivationFunctionType
ALU = mybir.AluOpType
DR = mybir.MatmulPerfMode.DoubleRow
E4 = ml_dtypes.float8_e4m3

SIM_TIME_NS = None


def build_nc():
    global SIM_TIME_NS
    nc = bacc.Bacc(None, target_bir_lowering=False)
    d = {}
    d["x8T"] = nc.dram_tensor("x8T", [D, L], FP8, kind="ExternalInput")
    d["xqT"] = nc.dram_tensor("xqT", [D, LQ], F32, kind="ExternalInput")
    d["wkq8"] = nc.dram_tensor("wkq8", [P, NDT, 2, NDT, P], FP8,
                               kind="ExternalInput")
    d["wv8"] = nc.dram_tensor("wv8", [D, D], FP8, kind="ExternalInput")
    d["wo8h"] = nc.dram_tensor("wo8h", [P, NDT, NDT, P], FP8,
                               kind="ExternalInput")
    d["wgh8"] = nc.dram_tensor("wgh8", [P, NHT, 2, NDT, P], FP8,
                               kind="ExternalInput")
    d["wobc8"] = nc.dram_tensor("wobc8", [P, NDT, 2, NHT, P], FP8,
                                kind="ExternalInput")
    d["bout_row"] = nc.dram_tensor("bout_row", [1, D], BF16,
                                   kind="ExternalInput")
    d["yT"] = nc.dram_tensor("yT", [D, LQ], F32, kind="ExternalOutput")

    with tile.TileContext(nc, pool_alloc_mode="queue") as tc:
        _body(tc, nc, d)
        _, snap = tc.schedule_and_allocate()
        SIM_TIME_NS = snap.time
    nc.compile()
    return nc


def _body(tc, nc, d):
    x8Tr = d["x8T"].rearrange("(dt p) l -> p dt l", p=P)
    xqTr = d["xqT"].rearrange("(dt p) l -> p dt l", p=P)
    yTr = d["yT"].rearrange("(dt p) l -> p dt l", p=P)

    with ExitStack() as pp_ctx:
        pp = pp_ctx.enter_context(tc.tile_pool(name="persist", bufs=1))
        eps_t = pp.tile([1, 1], F32, tag="eps")
        bm10 = pp.tile([1, 1], F32, tag="bm10")
        bm9 = pp.tile([1, 1], F32, tag="bm9")
        bp4 = pp.tile([1, 1], F32, tag="bp4")
        ones_col = pp.tile([P, 1], BF16, tag="ones")
        ones_row = pp.tile([1, 512], BF16, tag="onesr")
        bout_sb = pp.tile([1, D], BF16, tag="bout")
        x1T = pp.tile([P, NDT, LQ], F32, tag="x1T")
        x1n_o = pp_ctx.enter_context(tc.tile_pool(name="x1n", bufs=2))
        nc.vector.memset(eps_t, EPS)
        nc.vector.memset(bm10, -10.0 * LN2)
        nc.vector.memset(bm9, -9.0 * LN2)
        nc.vector.memset(bp4, 4.0 * LN2)
        nc.vector.memset(ones_col, 1.0)
        nc.vector.memset(ones_row, 1.0)
        nc.sync.dma_start(out=bout_sb, in_=d["bout_row"][:, :])
        x1ns = []
        ghq_sets = []

        gpsp = hpsp = None  # created after slice-0 attention (PSUM budget)

        def ffn_gh(ns, ghq8, ghqr8, wfp, tsp, gbp, pools):
            """gate/hidden (2-pass over x1n8/x1nr8) + silu chain -> gh tiles."""
            x1n8, x1nr8 = x1ns[ns]
            for jj in range(NHT // 2):
                wgh2 = wfp.tile([P, 2, 2, NDT, P], FP8, tag="wgh2")
                nc.sync.dma_start(out=wgh2,
                                  in_=d["wgh8"][:, 2 * jj:2 * jj + 2, :, :, :])
                for j in range(2):
                    ht = 2 * jj + j
                    gpool, hpool = pools[ht % len(pools)]
                    g_ps = gpool.tile([P, 512], F32, tag="g")
                    for dp in range(NDT // 2):
                        s2 = slice(2 * dp, 2 * dp + 2)
                        nc.tensor.matmul(g_ps, wgh2[:, j, 0, s2, :],
                                         x1n8[:, s2, :],
                                         start=(dp == 0), stop=False,
                                         perf_mode=DR)
                    for dp in range(NDT // 2):
                        s2 = slice(2 * dp, 2 * dp + 2)
                        nc.tensor.matmul(g_ps, wgh2[:, j, 0, s2, :],
                                         x1nr8[:, s2, :],
                                         start=False, stop=(dp == NDT // 2 - 1),
                                         perf_mode=DR)
                    h_ps = hpool.tile([P, 512], F32, tag="h")
                    for dp in range(NDT // 2):
                        s2 = slice(2 * dp, 2 * dp + 2)
                        nc.tensor.matmul(h_ps, wgh2[:, j, 1, s2, :],
                                         x1n8[:, s2, :],
                                         start=(dp == 0), stop=False,
                                         perf_mode=DR)
                    for dp in range(NDT // 2):
                        s2 = slice(2 * dp, 2 * dp + 2)
                        nc.tensor.matmul(h_ps, wgh2[:, j, 1, s2, :],
                                         x1nr8[:, s2, :],
                                         start=False, stop=(dp == NDT // 2 - 1),
                                         perf_mode=DR)
                    # silu(g)*h via tanh: t = tanh(G/2); gh = 0.5*G*(1+t)*H
                    t_sb = tsp.tile([P, 512], F32, tag="tanh")
                    nc.scalar.activation(t_sb, g_ps, AF.Tanh, scale=2.0 ** -15)
                    tmp = tsp.tile([P, 512], F32, tag="tmp")
                    nc.vector.scalar_tensor_tensor(
                        out=tmp, in0=t_sb, scalar=1.0, in1=g_ps,
                        op0=ALU.add, op1=ALU.mult)
                    gh_bf = gbp.tile([P, 512], BF16, tag="ghbf")
                    nc.vector.scalar_tensor_tensor(
                        out=gh_bf, in0=tmp, scalar=2.0 ** -25, in1=h_ps,
                        op0=ALU.mult, op1=ALU.mult)
                    nc.gpsimd.tensor_scalar(out=ghq8[:, ht, :], in0=gh_bf,
                                            scalar1=1.0, scalar2=None,
                                            op0=ALU.mult)
                    nc.gpsimd.tensor_sub(ghqr8[:, ht, :], gh_bf,
                                         ghq8[:, ht, :])

        def ffn_out(fpp, wop, finp):
            """out-projection 3-pass, fo-outer over both slices."""
            ybufs = [finp.tile([P, NDT, 512], F32, tag="yb0", name="yb0"),
                     finp.tile([P, NDT, 512], F32, tag="yb1", name="yb1")]
            for fo in range(NDT):
                wobc = wop.tile([P, 2, NHT, P], FP8, tag="wobc")
                nc.sync.dma_start(out=wobc, in_=d["wobc8"][:, fo, :, :, :])
                for ns in range(2):
                    ghq8, ghqr8 = ghq_sets[ns]
                    qsl = slice(ns * 512, (ns + 1) * 512)
                    fp = fpp.tile([P, 512], F32, tag="fp")
                    for hp in range(NHT // 2):
                        s2 = slice(2 * hp, 2 * hp + 2)
                        nc.tensor.matmul(fp, wobc[:, 0, s2, :], ghq8[:, s2, :],
                                         start=(hp == 0), stop=False,
                                         perf_mode=DR)
                    for hp in range(NHT // 2):
                        s2 = slice(2 * hp, 2 * hp + 2)
                        nc.tensor.matmul(fp, wobc[:, 0, s2, :], ghqr8[:, s2, :],
                                         start=False, stop=False, perf_mode=DR)
                    for hp in range(NHT // 2):
                        s2 = slice(2 * hp, 2 * hp + 2)
                        nc.tensor.matmul(fp, wobc[:, 1, s2, :], ghq8[:, s2, :],
                                         start=False, stop=False, perf_mode=DR)
                    # + b_out (scaled 2^14) via rank-1 bf16 matmul
                    nc.tensor.matmul(fp, bout_sb[:, fo * P:(fo + 1) * P],
                                     ones_row, start=False, stop=True)
                    nc.vector.scalar_tensor_tensor(
                        out=ybufs[ns][:, fo, :], in0=fp, scalar=2.0 ** -14,
                        in1=x1T[:, fo, qsl], op0=ALU.mult, op1=ALU.add)
            for ns in range(2):
                qsl = slice(ns * 512, (ns + 1) * 512)
                nc.gpsimd.dma_start(out=yTr[:, :, qsl], in_=ybufs[ns])

        ghp = pp_ctx.enter_context(tc.tile_pool(name="ghq", bufs=1))
        with ExitStack() as actx:
            ap = actx.enter_context(tc.tile_pool(name="attn", bufs=1))
            vt = ap.tile([P, NKT, NH, HD + 1], FP8, tag="vt")
            attnT = ap.tile([P, NDT, 512], FP8, tag="attnT")
            wo_all = ap.tile([P, NDT, NDT, P], FP8, tag="wo_all")
            kdrp = actx.enter_context(
                tc.tile_pool(name="kdr", bufs=1, space="DRAM"))
            kdrs = [kdrp.tile([P, L], FP8, tag=f"kdr{i}", name=f"kdr{i}")
                    for i in range(NDT)]
            qdrs = [kdrp.tile([P, LQ], FP8, tag=f"qdr{i}", name=f"qdr{i}")
                    for i in range(NDT)]
            nc.vector.memset(vt[:, :, :, HD:HD + 1], SV / SA)
            nc.sync.dma_start(out=wo_all, in_=d["wo8h"][:, :, :, :])

            # ---- P0: load x8, rmsnorm stats ----
            with ExitStack() as pctx:
                xp = pctx.enter_context(tc.tile_pool(name="xp", bufs=1))
                n1p = pctx.enter_context(tc.tile_pool(name="n1", bufs=3))
                bcp = pctx.enter_context(tc.tile_pool(name="bc1", bufs=2))
                bcP = pctx.enter_context(tc.tile_pool(name="bcP", bufs=1))
                rscp = pctx.enter_context(
                    tc.tile_pool(name="rsc", bufs=1, space="DRAM"))
                ssp = pctx.enter_context(
                    tc.tile_pool(name="ss1", bufs=2, space="PSUM"))
                prp = pctx.enter_context(
                    tc.tile_pool(name="proj", bufs=4, space="PSUM"))

                x8 = xp.tile([P, NDT, L], FP8, tag="x8")
                bck_all = bcP.tile([P, L // 512, 512], F32, tag="bck")
                bcq_all = bcP.tile([P, LQ // 512, 512], F32, tag="bcq")
                for ls in range(L // 512):
                    sl = slice(ls * 512, (ls + 1) * 512)
                    nc.gpsimd.dma_start(out=x8[:, :, sl], in_=x8Tr[:, :, sl])
                rsc = rscp.tile([L], F32, tag="rsc")
                for ls in range(L // 512):
                    sl = slice(ls * 512, (ls + 1) * 512)
                    ss_ps = ssp.tile([1, 512], F32, tag="ss")
                    for dt_ in range(NDT):
                        sq = n1p.tile([P, 512], BF16, tag="sq")
                        nc.gpsimd.tensor_mul(sq, x8[:, dt_, sl], x8[:, dt_, sl])
                        nc.tensor.matmul(ss_ps, ones_col, sq,
                                         start=(dt_ == 0), stop=(dt_ == NDT - 1))
                    lnr = bcp.tile([1, 512], F32, tag="lnr")
                    nc.scalar.activation(lnr, ss_ps, AF.Ln,
                                         bias=eps_t, scale=2.0 ** -18)
                    # rr_k = rr * 2^-10  (K evac, V evac);  rr_q = rr * 2^-9
                    rrk = bcp.tile([1, 512], F32, tag="rrk")
                    nc.scalar.activation(rrk, lnr, AF.Exp, scale=-0.5,
                                         bias=bm10)
                    nc.gpsimd.partition_broadcast(bck_all[:, ls, :], rrk)
                    nc.sync.dma_start(out=rsc[sl], in_=rrk)
                    if ls < LQ // 512:
                        rrq = bcp.tile([1, 512], F32, tag="rrq")
                        nc.scalar.activation(rrq, lnr, AF.Exp, scale=-0.5,
                                             bias=bm9)
                        nc.gpsimd.partition_broadcast(bcq_all[:, ls, :], rrq)
                rcol = bcP.tile([P, NKT], F32, tag="rcol")
                nc.sync.dma_start(out=rcol,
                                  in_=rsc.rearrange("(tt p) -> p tt", p=P))

                # ---- P1: K/Q/V projections (fp8 DR), DRAM round-trip ----
                with tc.tile_pool(name="wblk", bufs=2) as wp, \
                     tc.tile_pool(name="kmid", bufs=2) as kmp:
                    for ft in range(NDT):
                        wkq = wp.tile([P, 2, NDT, P], FP8, tag="wkq")
                        nc.sync.dma_start(out=wkq, in_=d["wkq8"][:, ft, :, :, :])
                        kmK = kmp.tile([P, L], FP8, tag="kmK")
                        for ks in range(L // 512):
                            sl = slice(ks * 512, (ks + 1) * 512)
                            ps = prp.tile([P, 512], F32, tag="pp")
                            for dp in range(NDT // 2):
                                s2 = slice(2 * dp, 2 * dp + 2)
                                nc.tensor.matmul(
                                    ps, wkq[:, 0, s2, :], x8[:, s2, sl],
                                    start=(dp == 0),
                                    stop=(dp == NDT // 2 - 1), perf_mode=DR)
                            nc.vector.tensor_mul(kmK[:, sl], ps,
                                                 bck_all[:, ks, :])
                        nc.sync.dma_start(out=kdrs[ft][:, :], in_=kmK)
                        kmQ = kmp.tile([P, LQ], FP8, tag="kmQ")
                        for ks in range(LQ // 512):
                            sl = slice(ks * 512, (ks + 1) * 512)
                            ps = prp.tile([P, 512], F32, tag="pp")
                            for dp in range(NDT // 2):
                                s2 = slice(2 * dp, 2 * dp + 2)
                                nc.tensor.matmul(
                                    ps, wkq[:, 1, s2, :], x8[:, s2, sl],
                                    start=(dp == 0),
                                    stop=(dp == NDT // 2 - 1), perf_mode=DR)
                            nc.vector.tensor_mul(kmQ[:, sl], ps,
                                                 bcq_all[:, ks, :])
                        nc.sync.dma_start(out=qdrs[ft][:, :], in_=kmQ)
                    # V: token-major psum, ACT evac with per-token scale
                    wvr = d["wv8"].rearrange("(dt p) f -> p dt f", p=P)
                    for hf in range(2):
                        wv = wp.tile([P, NDT, 512], FP8, tag="wv")
                        nc.sync.dma_start(
                            out=wv, in_=wvr[:, :, hf * 512:(hf + 1) * 512])
                        for tt in range(NKT):
                            ps = prp.tile([P, 512], F32, tag="pp")
                            for dp in range(NDT // 2):
                                s2 = slice(2 * dp, 2 * dp + 2)
                                nc.tensor.matmul(
                                    ps, x8[:, s2, tt * P:(tt + 1) * P],
                                    wv[:, s2, :],
                                    start=(dp == 0),
                                    stop=(dp == NDT // 2 - 1), perf_mode=DR)
                            nc.scalar.activation(
                                vt[:, tt, hf * 8:(hf + 1) * 8, 0:HD],
                                ps.rearrange("p (h e) -> p h e", h=8),
                                AF.Copy, scale=rcol[:, tt:tt + 1])

            # repartitioned K/Q views (per 2-head ft group):
            # within ft: partition p = h2*64 + hi*32 + lo ; head = 2*ft + h2
            kres = [t.rearrange("(h2 hi lo) k -> lo h2 hi k", h2=2, hi=2, lo=32)
                    for t in kdrs]
            qres = [t.rearrange("(h2 hi lo) k -> lo h2 hi k", h2=2, hi=2, lo=32)
                    for t in qdrs]

            # ---- P2: attention per 512-query slice ----
            with ExitStack() as ectx:
                kthp = ectx.enter_context(tc.tile_pool(name="kth", bufs=2))
                ptp = ectx.enter_context(tc.tile_pool(name="pt", bufs=8))
                smp = ectx.enter_context(tc.tile_pool(name="sm", bufs=2))
                rbp = ectx.enter_context(tc.tile_pool(name="rb", bufs=2))
                xqp = ectx.enter_context(tc.tile_pool(name="xq", bufs=1))
                sqp = ectx.enter_context(tc.tile_pool(name="sq2p", bufs=2))
                tsp0 = ectx.enter_context(tc.tile_pool(name="ts0", bufs=3))
                gbp0 = ectx.enter_context(tc.tile_pool(name="gb0", bufs=4))
                wfp0 = ectx.enter_context(tc.tile_pool(name="wf0", bufs=2))
                x1bp = ectx.enter_context(tc.tile_pool(name="x1b", bufs=3))
                gcount = 0
                for ns in range(2):
                    qsl = slice(ns * 512, (ns + 1) * 512)
                    # per-slice PSUM scope: slice 0 runs a depth-3 scores
                    # ring (6 banks) + 2 acc banks; slice 1 depth-2 + the
                    # gate/hidden overlap banks
                    sl_cm = ExitStack()
                    stp = sl_cm.enter_context(tc.tile_pool(
                        name=f"st{ns}", bufs=(3 if ns == 0 else 2),
                        space="PSUM"))
                    accp = sl_cm.enter_context(tc.tile_pool(
                        name=f"acc{ns}", bufs=2, space="PSUM"))
                    xqs = xqp.tile([P, NDT, 512], F32, tag="xqs")
                    nc.sync.dma_start(out=xqs, in_=xqTr[:, :, qsl])
                    for ft in range(NDT):
                        kth = kthp.tile([32, 2, 2, L], FP8, tag="kth")
                        nc.sync.dma_start(out=kth, in_=kres[ft])
                        qth = kthp.tile([32, 2, 2, 512], FP8, tag="qth")
                        nc.sync.dma_start(out=qth, in_=qres[ft][:, :, :, qsl])
                        # two parallel per-head chains: exp(h2=0) on ACT,
                        # exp(h2=1) mostly on DVE, so the score->exp->attnV
                        # chains advance concurrently on separate engines
                        accs = [accp.tile([HD + 1, 512], F32, tag="acc",
                                          name=f"acc{ns}_{ft}_{h2}")
                                for h2 in range(2)]
                        for g in range(NKT // 2):
                            for h2 in range(2):
                                h = 2 * ft + h2
                                acc = accs[h2]
                                st = stp.tile([P, 2, 512], F32, tag="st")
                                for j in range(2):
                                    kt = 2 * g + j
                                    nc.tensor.matmul(
                                        st[:, j, :],
                                        kth[:, h2, :, kt * P:(kt + 1) * P],
                                        qth[:, h2, :, :],
                                        start=True, stop=True, perf_mode=DR)
                                pt = ptp.tile([P, 2, 512], FP8, tag="pt")
                                use_dve = (h2 == 1) if ns == 0 else                                     (h2 == 1 and g % 2 == 1)
                                if use_dve:
                                    nc.vector.tensor_scalar(
                                        out=pt.bitcast(U8), in0=st,
                                        scalar1=K2B, scalar2=BCONST,
                                        op0=ALU.mult, op1=ALU.add)
                                else:
                                    nc.scalar.activation(
                                        pt, st, AF.Exp, scale=1.0 / (SKQ * SQ2))
                                nc.tensor.matmul(
                                    acc, vt[:, 2 * g:2 * g + 2, h, :], pt,
                                    start=(g == 0), stop=(g == NKT // 2 - 1),
                                    perf_mode=DR)
                        for h2 in range(2):
                            r0 = h2 * HD
                            acc = accs[h2]
                            rrow = smp.tile([1, 512], F32, tag="row")
                            nc.vector.reciprocal(rrow, acc[HD:HD + 1, :])
                            rb = rbp.tile([HD, 512], F32, tag="rb")
                            nc.gpsimd.partition_broadcast(rb, rrow)
                            nc.vector.tensor_mul(
                                attnT[r0:r0 + HD, ft, :], acc[0:HD, :], rb)

                    sl_cm.close()
                    if ns == 0:
                        gpsp = pp_ctx.enter_context(
                            tc.tile_pool(name="gps", bufs=1, space="PSUM"))
                        hpsp = pp_ctx.enter_context(
                            tc.tile_pool(name="hps", bufs=1, space="PSUM"))
                    # Wo projection (fp8 DR) + residual -> x1T
                    for ft in range(NDT):
                        ps = hpsp.tile([P, 512], F32, tag="h")
                        for dp in range(NDT // 2):
                            s2 = slice(2 * dp, 2 * dp + 2)
                            nc.tensor.matmul(
                                ps, wo_all[:, ft, s2, :], attnT[:, s2, :],
                                start=(dp == 0), stop=(dp == NDT // 2 - 1),
                                perf_mode=DR)
                        nc.vector.scalar_tensor_tensor(
                            out=x1T[:, ft, qsl], in0=ps, scalar=2.0 ** -20,
                            in1=xqs[:, ft, :], op0=ALU.mult, op1=ALU.add)

                    # rmsnorm2 -> x1n8 + x1nr8 (2-term fp8, Pool chain)
                    ss2t = gpsp.tile([P, 512], F32, tag="g")
                    ss2 = ss2t[0:1, :]
                    for dt_ in range(NDT):
                        sq2 = sqp.tile([P, 512], BF16, tag="sq2")
                        nc.gpsimd.tensor_mul(sq2, x1T[:, dt_, qsl],
                                             x1T[:, dt_, qsl])
                        nc.tensor.matmul(ss2, ones_col, sq2,
                                         start=(dt_ == 0), stop=(dt_ == NDT - 1))
                    ln2 = smp.tile([1, 512], F32, tag="row")
                    nc.scalar.activation(ln2, ss2, AF.Ln, bias=eps_t,
                                         scale=1.0 / D)
                    rr2 = smp.tile([1, 512], F32, tag="row")
                    nc.scalar.activation(rr2, ln2, AF.Exp, scale=-0.5,
                                         bias=bp4)
                    bc2 = rbp.tile([P, 512], F32, tag="rb2")
                    nc.gpsimd.partition_broadcast(bc2, rr2)
                    x1n8 = x1n_o.tile([P, NDT, 512], FP8, tag="x1n8")
                    x1nr8 = x1n_o.tile([P, NDT, 512], FP8, tag="x1nr8")
                    for dt_ in range(NDT):
                        x1b = x1bp.tile([P, 512], BF16, tag="x1b")
                        nc.gpsimd.tensor_mul(x1b, x1T[:, dt_, qsl], bc2)
                        nc.gpsimd.tensor_scalar(out=x1n8[:, dt_, :], in0=x1b,
                                                scalar1=1.0, scalar2=None,
                                                op0=ALU.mult)
                        nc.gpsimd.tensor_sub(x1nr8[:, dt_, :], x1b,
                                             x1n8[:, dt_, :])
                    x1ns.append((x1n8, x1nr8))
                    if ns == 0:
                        ghq8 = ghp.tile([P, NHT, 512], FP8, tag="gh8")
                        ghqr8 = ghp.tile([P, NHT, 512], FP8, tag="ghr8")
                        ghq_sets.append((ghq8, ghqr8))
                        ffn_gh(0, ghq8, ghqr8, wfp0, tsp0, gbp0,
                               [(gpsp, hpsp)])
            # attention pools closed: PSUM st/acc freed

        # ---- P3: gate/hidden(1), then fo-outer out-proj over both slices ----
        with ExitStack() as fctx:
            fpp = fctx.enter_context(
                tc.tile_pool(name="fpp", bufs=2, space="PSUM"))
            gpsp2 = fctx.enter_context(
                tc.tile_pool(name="gps2", bufs=1, space="PSUM"))
            hpsp2 = fctx.enter_context(
                tc.tile_pool(name="hps2", bufs=1, space="PSUM"))
            tsp = fctx.enter_context(tc.tile_pool(name="tsb", bufs=3))
            gbp = fctx.enter_context(tc.tile_pool(name="gb1", bufs=4))
            wfp = fctx.enter_context(tc.tile_pool(name="wffn", bufs=2))
            wop2 = fctx.enter_context(tc.tile_pool(name="wob2", bufs=2))
            finp = fctx.enter_context(tc.tile_pool(name="fin", bufs=1))

            ghpb = fctx.enter_context(tc.tile_pool(name="ghqb", bufs=1))
            ghq8b = ghpb.tile([P, NHT, 512], FP8, tag="gh8b")
            ghqr8b = ghpb.tile([P, NHT, 512], FP8, tag="ghr8b")
            ghq_sets.append((ghq8b, ghqr8b))
            ffn_gh(1, ghq8b, ghqr8b, wfp, tsp, gbp,
                   [(gpsp, hpsp), (gpsp2, hpsp2)])
            ffn_out(fpp, wop2, finp)


_NC_CACHE = {}


def kernel(x, W_q, W_k, W_v, W_o, b_o, attn_norm_w, ffn_norm_w,
           W_gate, W_hidden, W_out, b_out):
    x = np.asarray(x, np.float32)
    w1 = np.asarray(attn_norm_w, np.float32)[:, None]
    w2 = np.asarray(ffn_norm_w, np.float32)[:, None]

    def q8(a, sc):
        return (np.ascontiguousarray(np.asarray(a, np.float32)) * sc).astype(E4)

    def blk4(a):
        """[D, F] -> [P(p), F//128(ft), D//128(dt), 128] weight layout."""
        dd, ff = a.shape
        return np.ascontiguousarray(
            a.reshape(dd // P, P, ff // P, P).transpose(1, 2, 0, 3))

    wq8 = blk4(q8(np.asarray(W_q, np.float32).T * w1 / np.sqrt(HD), SWQ))
    wk8 = blk4(q8(np.asarray(W_k, np.float32).T * w1, SWK))
    wkq8 = np.ascontiguousarray(np.stack([wk8, wq8], axis=2))
    wv8 = q8(np.asarray(W_v, np.float32).T * w1, SWV)
    wo8h = blk4(q8(np.asarray(W_o, np.float32).T, SWO))
    wg8 = blk4(q8(np.asarray(W_gate, np.float32).T * w2, SWF))
    wh8 = blk4(q8(np.asarray(W_hidden, np.float32).T * w2, SWF))
    wgh8 = np.ascontiguousarray(np.stack([wg8, wh8], axis=2))
    wobf = np.ascontiguousarray(np.asarray(W_out, np.float32).T) * SWF
    wob8 = wobf.astype(E4)
    wobr8 = (wobf - wob8.astype(np.float32)).astype(E4)
    wobc8 = np.ascontiguousarray(
        np.stack([blk4(wob8), blk4(wobr8)], axis=2))
    bout_row = np.ascontiguousarray(
        (np.asarray(b_out, np.float32) * (SWF * SGH))[None, :]
    ).astype(ml_dtypes.bfloat16)
    bo = np.asarray(b_o, np.float32)

    if "nc" not in _NC_CACHE:
        _NC_CACHE["nc"] = build_nc()
    nc = _NC_CACHE["nc"]

    in_maps = []
    for c in range(8):
        b, half = c // 2, c % 2
        xb = x[b]
        if half:
            xb = np.concatenate([xb[LQ:], xb[:LQ]], axis=0)
        in_maps.append({
            "x8T": np.ascontiguousarray((xb.T * SX1)).astype(E4),
            "xqT": np.ascontiguousarray(xb[:LQ].T + bo[:, None]),
            "wkq8": wkq8, "wv8": wv8, "wo8h": wo8h,
            "wgh8": wgh8, "wobc8": wobc8, "bout_row": bout_row,
        })
    res = bass_utils.run_bass_kernel_spmd(nc, in_maps, core_ids=list(range(8)))
    y = np.empty((4, L, D), np.float32)
    for c in range(8):
        b, half = c // 2, c % 2
        y[b, half * LQ:(half + 1) * LQ, :] = res.results[c]["yT"].T
    return y


# revision 41
# speedup vs baseline: 1.6562x; 1.0181x over previous
"""Dense transformer block (RMSNorm+MHA+residual, RMSNorm+SwiGLU+residual)
on 8 trn2 NeuronCores. Sharding: 2 cores per batch element; each core
computes the block output for 1024 of its batch's 2048 tokens, redundantly
computing K/V for the full sequence (keys are permutation invariant; each
core's x puts its own 1024 query tokens first). No inter-core communication.

fp8 (e4m3) DoubleRow design: every large matmul runs fp8 with
MatmulPerfMode.DoubleRow (0.5 cycles/row, 256-deep contraction pairs).
Numerics validated in numpy + on HW (max abs err ~0.07 vs gate 0.109):
- attention path entirely 1-term fp8 (x, wq/wk/wv/wo, k/q/v, probs, attn)
  with power-of-2 scales; rmsnorm scales folded into PSUM evacuations and
  host-prefolded weights.
- FFN: 1-term fp8 weights x 2-term (flat-scale residual) fp8 x1n for
  gate/hidden; out-proj 3-pass (w8*gh8 + w8*ghr8 + wr8*gh8) with the
  2-term gh split computed on Pool (cast + subtract from a bf16 master).
- softmax exp split between ACT (table exp -> fp8) and DVE (Schraudolph
  uint8 bit-trick via tensor_scalar, bitcast into the same fp8 tile);
  softmax normalization cancels the bit-trick's systematic error.
- scores use a repartitioned K/Q layout [32(lo), 2(h2), 2(hi), tokens]
  per 2-head group, produced via a DRAM round-trip, so DoubleRow can pair
  the two 32-feature halves of each 64-wide head.
- silu via tanh identity keeps ACT on the exp-compatible table set;
  FFN(slice0) gate/hidden matmuls overlap the slice-1 attention window.
- DMA discipline: host-side weight layouts give >=1KB contiguous runs and
  one DMA per tile group (~150 DMAs total); HWDGE triggers on the
  otherwise-idle SP engine (each holds SEQ+HWDGE ~700ns), bulk x8/y on
  gpsimd SWDGE.
"""
import sys
from contextlib import ExitStack

import numpy as np

sys.path.insert(0, "/opt/trn_rl_repo")

import ml_dtypes  # noqa: E402
import concourse.bass as bass  # noqa: E402
from concourse import bacc  # noqa: E402
import concourse.tile as tile  # noqa: E402
from concourse import mybir  # noqa: E402
from concourse import bass_utils  # noqa: E402

P = 128
D = 1024          # d_model
L = 2048          # full seq per core (keys)
LQ = 1024         # query tokens per core
NH = 16
HD = 64
HID = 4096
EPS = 1e-6
NDT = D // P      # 8 feature tiles
NKT = L // P      # 16 key tiles
NHT = HID // P    # 32 hidden tiles
LN2 = float(np.log(2.0))

# power-of-2 fp8 scales (validated in acc_sim.py)
SX1 = 16.0        # x8 = fp8(x * SX1)
SWQ, SWK, SWV, SWO = 8192.0, 2048.0, 2048.0, 1024.0
SKQ, SQ2, SV, SA = 32.0, 256.0, 32.0, 1024.0
SX2, SWF, SGH = 16.0, 1024.0, 16.0
# Schraudolph exp on DVE: uint8 bits = st*K2B + BCONST, bitcast to e4m3
K2B = float(8.0 * np.log2(np.e) / (SKQ * SQ2))   # st = 8192 * s_true
BCONST = 55.5                                     # 7*8 + c_adj(-0.5)
EXP_DVE_MOD = 4   # every 4th exp group goes to DVE

F32 = mybir.dt.float32
BF16 = mybir.dt.bfloat16
FP8 = mybir.dt.float8e4
U8 = mybir.dt.uint8
AF = mybir.ActivationFunctionType
ALU = mybir.AluOpType
DR = mybir.MatmulPerfMode.DoubleRow
E4 = ml_dtypes.float8_e4m3

SIM_TIME_NS = None


def build_nc():
    global SIM_TIME_NS
    nc = bacc.Bacc(None, target_bir_lowering=False)
    d = {}
    d["x8T"] = nc.dram_tensor("x8T", [D, L], FP8, kind="ExternalInput")
    d["xqT"] = nc.dram_tensor("xqT", [D, LQ], BF16, kind="ExternalInput")
    d["wkq8"] = nc.dram_tensor("wkq8", [P, NDT, 2, NDT, P], FP8,
                               kind="ExternalInput")
    d["wv8"] = nc.dram_tensor("wv8", [D, D], FP8, kind="ExternalInput")
    d["wo8h"] = nc.dram_tensor("wo8h", [P, NDT, NDT, P], FP8,
                               kind="ExternalInput")
    d["wgh8"] = nc.dram_tensor("wgh8", [P, NHT, 2, NDT, P], FP8,
                               kind="ExternalInput")
    d["wobc8"] = nc.dram_tensor("wobc8", [P, NDT, 2, NHT, P], FP8,
                                kind="ExternalInput")
    d["bout_row"] = nc.dram_tensor("bout_row", [1, D], BF16,
                                   kind="ExternalInput")
    d["yT"] = nc.dram_tensor("yT", [D, LQ], F32, kind="ExternalOutput")

    with tile.TileContext(nc) as tc:
        _body(tc, nc, d)
        _, snap = tc.schedule_and_allocate()
        SIM_TIME_NS = snap.time
    nc.compile()
    return nc


def _body(tc, nc, d):
    x8Tr = d["x8T"].rearrange("(dt p) l -> p dt l", p=P)
    xqTr = d["xqT"].rearrange("(dt p) l -> p dt l", p=P)
    yTr = d["yT"].rearrange("(dt p) l -> p dt l", p=P)

    with ExitStack() as pp_ctx:
        pp = pp_ctx.enter_context(tc.tile_pool(name="persist", bufs=1))
        eps_t = pp.tile([1, 1], F32, tag="eps")
        bm10 = pp.tile([1, 1], F32, tag="bm10")
        bm9 = pp.tile([1, 1], F32, tag="bm9")
        bp4 = pp.tile([1, 1], F32, tag="bp4")
        ones_col = pp.tile([P, 1], BF16, tag="ones")
        ones_row = pp.tile([1, 512], BF16, tag="onesr")
        bout_sb = pp.tile([1, D], BF16, tag="bout")
        x1T = pp.tile([P, NDT, LQ], F32, tag="x1T")
        x1n_o = pp_ctx.enter_context(tc.tile_pool(name="x1n", bufs=2))
        nc.vector.memset(eps_t, EPS)
        nc.vector.memset(bm10, -10.0 * LN2)
        nc.vector.memset(bm9, -9.0 * LN2)
        nc.vector.memset(bp4, 4.0 * LN2)
        nc.vector.memset(ones_col, 1.0)
        nc.vector.memset(ones_row, 1.0)
        nc.sync.dma_start(out=bout_sb, in_=d["bout_row"][:, :])
        x1ns = []
        ghq_sets = []

        gpsp = hpsp = None  # created after slice-0 attention (PSUM budget)

        def ffn_gh_jj(ns, jj, ghq8, ghqr8, wfp, tsp, gbp, pools):
            """gate/hidden 2-ht group (2-pass over x1n8/x1nr8) + silu chain."""
            x1n8, x1nr8 = x1ns[ns]
            wgh2 = wfp.tile([P, 2, 2, NDT, P], FP8, tag="wgh2")
            nc.sync.dma_start(out=wgh2,
                              in_=d["wgh8"][:, 2 * jj:2 * jj + 2, :, :, :])
            for j in range(2):
                ht = 2 * jj + j
                gpool, hpool = pools[ht % len(pools)]
                g_ps = gpool.tile([P, 512], F32, tag="g")
                for dp in range(NDT // 2):
                    s2 = slice(2 * dp, 2 * dp + 2)
                    nc.tensor.matmul(g_ps, wgh2[:, j, 0, s2, :],
                                     x1n8[:, s2, :],
                                     start=(dp == 0), stop=False,
                                     perf_mode=DR)
                for dp in range(NDT // 2):
                    s2 = slice(2 * dp, 2 * dp + 2)
                    nc.tensor.matmul(g_ps, wgh2[:, j, 0, s2, :],
                                     x1nr8[:, s2, :],
                                     start=False, stop=(dp == NDT // 2 - 1),
                                     perf_mode=DR)
                h_ps = hpool.tile([P, 512], F32, tag="h")
                for dp in range(NDT // 2):
                    s2 = slice(2 * dp, 2 * dp + 2)
                    nc.tensor.matmul(h_ps, wgh2[:, j, 1, s2, :],
                                     x1n8[:, s2, :],
                                     start=(dp == 0), stop=False,
                                     perf_mode=DR)
                for dp in range(NDT // 2):
                    s2 = slice(2 * dp, 2 * dp + 2)
                    nc.tensor.matmul(h_ps, wgh2[:, j, 1, s2, :],
                                     x1nr8[:, s2, :],
                                     start=False, stop=(dp == NDT // 2 - 1),
                                     perf_mode=DR)
                # silu(g)*h via tanh: t = tanh(G/2); gh = 0.5*G*(1+t)*H
                t_sb = tsp.tile([P, 512], F32, tag="tanh")
                nc.scalar.activation(t_sb, g_ps, AF.Tanh, scale=2.0 ** -15)
                tmp = tsp.tile([P, 512], F32, tag="tmp")
                nc.vector.scalar_tensor_tensor(
                    out=tmp, in0=t_sb, scalar=1.0, in1=g_ps,
                    op0=ALU.add, op1=ALU.mult)
                gh_bf = gbp.tile([P, 512], BF16, tag="ghbf")
                nc.vector.scalar_tensor_tensor(
                    out=gh_bf, in0=tmp, scalar=2.0 ** -25, in1=h_ps,
                    op0=ALU.mult, op1=ALU.mult)
                nc.gpsimd.tensor_scalar(out=ghq8[:, ht, :], in0=gh_bf,
                                        scalar1=1.0, scalar2=None,
                                        op0=ALU.mult)
                nc.gpsimd.tensor_sub(ghqr8[:, ht, :], gh_bf,
                                     ghq8[:, ht, :])

        def ffn_out_fo(ns, fo, wop, fpp, ybuf):
            """out-projection 3-pass for one (slice, feature-block)."""
            ghq8, ghqr8 = ghq_sets[ns]
            qsl = slice(ns * 512, (ns + 1) * 512)
            wobc = wop.tile([P, 2, NHT, P], FP8, tag="wobc")
            nc.sync.dma_start(out=wobc, in_=d["wobc8"][:, fo, :, :, :])
            fp = fpp.tile([P, 512], F32, tag="fp")
            for hp in range(NHT // 2):
                s2 = slice(2 * hp, 2 * hp + 2)
                nc.tensor.matmul(fp, wobc[:, 0, s2, :], ghq8[:, s2, :],
                                 start=(hp == 0), stop=False, perf_mode=DR)
            for hp in range(NHT // 2):
                s2 = slice(2 * hp, 2 * hp + 2)
                nc.tensor.matmul(fp, wobc[:, 0, s2, :], ghqr8[:, s2, :],
                                 start=False, stop=False, perf_mode=DR)
            for hp in range(NHT // 2):
                s2 = slice(2 * hp, 2 * hp + 2)
                nc.tensor.matmul(fp, wobc[:, 1, s2, :], ghq8[:, s2, :],
                                 start=False, stop=False, perf_mode=DR)
            # + b_out (scaled 2^14) via rank-1 bf16 matmul
            nc.tensor.matmul(fp, bout_sb[:, fo * P:(fo + 1) * P],
                             ones_row, start=False, stop=True)
            nc.vector.scalar_tensor_tensor(
                out=ybuf[:, fo, :], in0=fp, scalar=2.0 ** -14,
                in1=x1T[:, fo, qsl], op0=ALU.mult, op1=ALU.add)

        ghp = pp_ctx.enter_context(tc.tile_pool(name="ghq", bufs=1))
        with ExitStack() as actx:
            ap = actx.enter_context(tc.tile_pool(name="attn", bufs=1))
            vt = ap.tile([P, NKT, NH, HD + 1], FP8, tag="vt")
            attnT = ap.tile([P, NDT, 512], FP8, tag="attnT")
            wo_all = ap.tile([P, NDT, NDT, P], FP8, tag="wo_all")
            kdrp = actx.enter_context(
                tc.tile_pool(name="kdr", bufs=1, space="DRAM"))
            kdrs = [kdrp.tile([P, L], FP8, tag=f"kdr{i}", name=f"kdr{i}")
                    for i in range(NDT)]
            qdrs = [kdrp.tile([P, LQ], FP8, tag=f"qdr{i}", name=f"qdr{i}")
                    for i in range(NDT)]
            nc.vector.memset(vt[:, :, :, HD:HD + 1], SV / SA)
            nc.sync.dma_start(out=wo_all, in_=d["wo8h"][:, :, :, :])

            # ---- P0: load x8, rmsnorm stats ----
            with ExitStack() as pctx:
                xp = pctx.enter_context(tc.tile_pool(name="xp", bufs=1))
                n1p = pctx.enter_context(tc.tile_pool(name="n1", bufs=3))
                bcp = pctx.enter_context(tc.tile_pool(name="bc1", bufs=2))
                bcP = pctx.enter_context(tc.tile_pool(name="bcP", bufs=1))
                rscp = pctx.enter_context(
                    tc.tile_pool(name="rsc", bufs=1, space="DRAM"))
                ssp = pctx.enter_context(
                    tc.tile_pool(name="ss1", bufs=2, space="PSUM"))
                prp = pctx.enter_context(
                    tc.tile_pool(name="proj", bufs=4, space="PSUM"))

                x8 = xp.tile([P, NDT, L], FP8, tag="x8")
                bck_all = bcP.tile([P, L // 512, 512], F32, tag="bck")
                bcq_all = bcP.tile([P, LQ // 512, 512], F32, tag="bcq")
                for ls in range(L // 512):
                    sl = slice(ls * 512, (ls + 1) * 512)
                    nc.gpsimd.dma_start(out=x8[:, :, sl], in_=x8Tr[:, :, sl])
                rsc = rscp.tile([L], F32, tag="rsc")
                for ls in range(L // 512):
                    sl = slice(ls * 512, (ls + 1) * 512)
                    ss_ps = ssp.tile([1, 512], F32, tag="ss")
                    for dt_ in range(NDT):
                        sq = n1p.tile([P, 512], BF16, tag="sq")
                        if dt_ % 2 == 0:
                            nc.gpsimd.tensor_mul(sq, x8[:, dt_, sl],
                                                 x8[:, dt_, sl])
                        else:
                            nc.vector.tensor_mul(sq, x8[:, dt_, sl],
                                                 x8[:, dt_, sl])
                        nc.tensor.matmul(ss_ps, ones_col, sq,
                                         start=(dt_ == 0), stop=(dt_ == NDT - 1))
                    lnr = bcp.tile([1, 512], F32, tag="lnr")
                    nc.scalar.activation(lnr, ss_ps, AF.Ln,
                                         bias=eps_t, scale=2.0 ** -18)
                    # rr_k = rr * 2^-10  (K evac, V evac);  rr_q = rr * 2^-9
                    rrk = bcp.tile([1, 512], F32, tag="rrk")
                    nc.scalar.activation(rrk, lnr, AF.Exp, scale=-0.5,
                                         bias=bm10)
                    nc.gpsimd.partition_broadcast(bck_all[:, ls, :], rrk)
                    nc.sync.dma_start(out=rsc[sl], in_=rrk)
                    if ls < LQ // 512:
                        rrq = bcp.tile([1, 512], F32, tag="rrq")
                        nc.scalar.activation(rrq, lnr, AF.Exp, scale=-0.5,
                                             bias=bm9)
                        nc.gpsimd.partition_broadcast(bcq_all[:, ls, :], rrq)
                rcol = bcP.tile([P, NKT], F32, tag="rcol")
                nc.sync.dma_start(out=rcol,
                                  in_=rsc.rearrange("(tt p) -> p tt", p=P))

                # ---- P1: K/Q/V projections (fp8 DR), DRAM round-trip ----
                with tc.tile_pool(name="wblk", bufs=2) as wp, \
                     tc.tile_pool(name="kmid", bufs=2) as kmp:
                    for ft in range(NDT):
                        wkq = wp.tile([P, 2, NDT, P], FP8, tag="wkq")
                        nc.sync.dma_start(out=wkq,
                                           in_=d["wkq8"][:, ft, :, :, :])
                        kmK = kmp.tile([P, L], FP8, tag="kmK")
                        for ks in range(L // 512):
                            sl = slice(ks * 512, (ks + 1) * 512)
                            ps = prp.tile([P, 512], F32, tag="pp")
                            for dp in range(NDT // 2):
                                s2 = slice(2 * dp, 2 * dp + 2)
                                nc.tensor.matmul(
                                    ps, wkq[:, 0, s2, :], x8[:, s2, sl],
                                    start=(dp == 0),
                                    stop=(dp == NDT // 2 - 1), perf_mode=DR)
                            nc.vector.tensor_mul(kmK[:, sl], ps,
                                                 bck_all[:, ks, :])
                        nc.sync.dma_start(out=kdrs[ft][:, :], in_=kmK)
                        kmQ = kmp.tile([P, LQ], FP8, tag="kmQ")
                        for ks in range(LQ // 512):
                            sl = slice(ks * 512, (ks + 1) * 512)
                            ps = prp.tile([P, 512], F32, tag="pp")
                            for dp in range(NDT // 2):
                                s2 = slice(2 * dp, 2 * dp + 2)
                                nc.tensor.matmul(
                                    ps, wkq[:, 1, s2, :], x8[:, s2, sl],
                                    start=(dp == 0),
                                    stop=(dp == NDT // 2 - 1), perf_mode=DR)
                            nc.vector.tensor_mul(kmQ[:, sl], ps,
                                                 bcq_all[:, ks, :])
                        nc.sync.dma_start(out=qdrs[ft][:, :], in_=kmQ)
                    # V: token-major psum, ACT evac with per-token scale
                    wvr = d["wv8"].rearrange("(dt p) f -> p dt f", p=P)
                    for hf in range(2):
                        wv = wp.tile([P, NDT, 512], FP8, tag="wv")
                        nc.sync.dma_start(
                            out=wv, in_=wvr[:, :, hf * 512:(hf + 1) * 512])
                        for tt in range(NKT):
                            ps = prp.tile([P, 512], F32, tag="pp")
                            for dp in range(NDT // 2):
                                s2 = slice(2 * dp, 2 * dp + 2)
                                nc.tensor.matmul(
                                    ps, x8[:, s2, tt * P:(tt + 1) * P],
                                    wv[:, s2, :],
                                    start=(dp == 0),
                                    stop=(dp == NDT // 2 - 1), perf_mode=DR)
                            if tt % 2 == 0:
                                nc.scalar.activation(
                                    vt[:, tt, hf * 8:(hf + 1) * 8, 0:HD],
                                    ps.rearrange("p (h e) -> p h e", h=8),
                                    AF.Copy, scale=rcol[:, tt:tt + 1])
                            else:
                                nc.vector.tensor_scalar(
                                    out=vt[:, tt, hf * 8:(hf + 1) * 8, 0:HD],
                                    in0=ps.rearrange("p (h e) -> p h e", h=8),
                                    scalar1=rcol[:, tt:tt + 1], scalar2=None,
                                    op0=ALU.mult)

            # repartitioned K/Q views (per 2-head ft group):
            # within ft: partition p = h2*64 + hi*32 + lo ; head = 2*ft + h2
            kres = [t.rearrange("(h2 hi lo) k -> lo h2 hi k", h2=2, hi=2, lo=32)
                    for t in kdrs]
            qres = [t.rearrange("(h2 hi lo) k -> lo h2 hi k", h2=2, hi=2, lo=32)
                    for t in qdrs]

            # small SBUF pools shared by attention + deferred Wo/norm
            smp = actx.enter_context(tc.tile_pool(name="sm", bufs=2))
            rbp = actx.enter_context(tc.tile_pool(name="rb", bufs=2))
            xqp = actx.enter_context(tc.tile_pool(name="xq", bufs=2))
            sqp = actx.enter_context(tc.tile_pool(name="sq2p", bufs=2))
            x1bp = actx.enter_context(tc.tile_pool(name="x1b", bufs=3))
            tsp0 = actx.enter_context(tc.tile_pool(name="ts0", bufs=3))
            gbp0 = actx.enter_context(tc.tile_pool(name="gb0", bufs=4))
            wfp0 = actx.enter_context(tc.tile_pool(name="wf0", bufs=2))
            xqs_l = []

            def emit_wo_norm(ns, wopool, wotag, sspool, sstag):
                qsl = slice(ns * 512, (ns + 1) * 512)
                # Wo projection (fp8 DR) + residual -> x1T
                for ft in range(NDT):
                    ps = wopool.tile([P, 512], F32, tag=wotag)
                    for dp in range(NDT // 2):
                        s2 = slice(2 * dp, 2 * dp + 2)
                        nc.tensor.matmul(
                            ps, wo_all[:, ft, s2, :], attnT[:, s2, :],
                            start=(dp == 0), stop=(dp == NDT // 2 - 1),
                            perf_mode=DR)
                    nc.vector.scalar_tensor_tensor(
                        out=x1T[:, ft, qsl], in0=ps, scalar=2.0 ** -20,
                        in1=xqs_l[ns][:, ft, :], op0=ALU.mult, op1=ALU.add)
                # rmsnorm2 -> x1n8 + x1nr8 (2-term fp8, Pool chain)
                ss2t = sspool.tile([P, 512], F32, tag=sstag)
                ss2 = ss2t[0:1, :]
                for dt_ in range(NDT):
                    sq2 = sqp.tile([P, 512], BF16, tag="sq2")
                    nc.gpsimd.tensor_mul(sq2, x1T[:, dt_, qsl],
                                         x1T[:, dt_, qsl])
                    nc.tensor.matmul(ss2, ones_col, sq2,
                                     start=(dt_ == 0), stop=(dt_ == NDT - 1))
                ln2 = smp.tile([1, 512], F32, tag="row")
                nc.scalar.activation(ln2, ss2, AF.Ln, bias=eps_t,
                                     scale=1.0 / D)
                rr2 = smp.tile([1, 512], F32, tag="row")
                nc.scalar.activation(rr2, ln2, AF.Exp, scale=-0.5,
                                     bias=bp4)
                bc2 = rbp.tile([P, 512], F32, tag="rb2")
                nc.gpsimd.partition_broadcast(bc2, rr2)
                x1n8 = x1n_o.tile([P, NDT, 512], FP8, tag="x1n8")
                x1nr8 = x1n_o.tile([P, NDT, 512], FP8, tag="x1nr8")
                for dt_ in range(NDT):
                    x1b = x1bp.tile([P, 512], BF16, tag="x1b")
                    nc.gpsimd.tensor_mul(x1b, x1T[:, dt_, qsl], bc2)
                    nc.gpsimd.tensor_scalar(out=x1n8[:, dt_, :], in0=x1b,
                                            scalar1=1.0, scalar2=None,
                                            op0=ALU.mult)
                    nc.gpsimd.tensor_sub(x1nr8[:, dt_, :], x1b,
                                         x1n8[:, dt_, :])
                x1ns.append((x1n8, x1nr8))

            # ---- P2: attention per 512-query slice ----
            with ExitStack() as ectx:
                kthp = ectx.enter_context(tc.tile_pool(name="kth", bufs=2))
                ptp = ectx.enter_context(tc.tile_pool(name="pt", bufs=8))
                gcount = 0
                for ns in range(2):
                    qsl = slice(ns * 512, (ns + 1) * 512)
                    # per-slice PSUM scope: slice 0 runs a depth-3 scores
                    # ring (6 banks) + 2 acc banks; slice 1 depth-2 + the
                    # gate/hidden overlap banks
                    sl_cm = ExitStack()
                    stp = sl_cm.enter_context(tc.tile_pool(
                        name=f"st{ns}", bufs=(3 if ns == 0 else 2),
                        space="PSUM"))
                    accp = sl_cm.enter_context(tc.tile_pool(
                        name=f"acc{ns}", bufs=2, space="PSUM"))
                    xqs = xqp.tile([P, NDT, 512], BF16, tag="xqs")
                    nc.sync.dma_start(out=xqs, in_=xqTr[:, :, qsl])
                    xqs_l.append(xqs)
                    for ft in range(NDT):
                        kth = kthp.tile([32, 2, 2, L], FP8, tag="kth")
                        nc.sync.dma_start(out=kth, in_=kres[ft])
                        qth = kthp.tile([32, 2, 2, 512], FP8, tag="qth")
                        nc.sync.dma_start(out=qth, in_=qres[ft][:, :, :, qsl])
                        # two parallel per-head chains: exp(h2=0) on ACT,
                        # exp(h2=1) mostly on DVE, so the score->exp->attnV
                        # chains advance concurrently on separate engines
                        accs = [accp.tile([HD + 1, 512], F32, tag="acc",
                                          name=f"acc{ns}_{ft}_{h2}")
                                for h2 in range(2)]
                        for g in range(NKT // 2):
                            for h2 in range(2):
                                h = 2 * ft + h2
                                acc = accs[h2]
                                st = stp.tile([P, 2, 512], F32, tag="st")
                                for j in range(2):
                                    kt = 2 * g + j
                                    nc.tensor.matmul(
                                        st[:, j, :],
                                        kth[:, h2, :, kt * P:(kt + 1) * P],
                                        qth[:, h2, :, :],
                                        start=True, stop=True, perf_mode=DR)
                                pt = ptp.tile([P, 2, 512], FP8, tag="pt")
                                use_dve = (
                                    (h2 == 1 and g % 4 != 3) if ns == 0
                                    else (h2 == 1 and g % 2 == 1))
                                if use_dve:
                                    nc.vector.tensor_scalar(
                                        out=pt.bitcast(U8), in0=st,
                                        scalar1=K2B, scalar2=BCONST,
                                        op0=ALU.mult, op1=ALU.add)
                                else:
                                    nc.scalar.activation(
                                        pt, st, AF.Exp, scale=1.0 / (SKQ * SQ2))
                                nc.tensor.matmul(
                                    acc, vt[:, 2 * g:2 * g + 2, h, :], pt,
                                    start=(g == 0), stop=(g == NKT // 2 - 1),
                                    perf_mode=DR)
                        for h2 in range(2):
                            r0 = h2 * HD
                            acc = accs[h2]
                            rrow = smp.tile([1, 512], F32, tag="row")
                            nc.vector.reciprocal(rrow, acc[HD:HD + 1, :])
                            rb = rbp.tile([HD, 512], F32, tag="rb")
                            nc.gpsimd.partition_broadcast(rb, rrow)
                            nc.vector.tensor_mul(
                                attnT[r0:r0 + HD, ft, :], acc[0:HD, :], rb)

                    sl_cm.close()
                    if ns == 0:
                        gpsp = pp_ctx.enter_context(
                            tc.tile_pool(name="gps", bufs=1, space="PSUM"))
                        hpsp = pp_ctx.enter_context(
                            tc.tile_pool(name="hps", bufs=1, space="PSUM"))
                        emit_wo_norm(0, hpsp, "h", gpsp, "g")
                        ghq8 = ghp.tile([P, NHT, 512], FP8, tag="gh8")
                        ghqr8 = ghp.tile([P, NHT, 512], FP8, tag="ghr8")
                        ghq_sets.append((ghq8, ghqr8))
                        for jj in range(NHT // 2):
                            ffn_gh_jj(0, jj, ghq8, ghqr8, wfp0, tsp0, gbp0,
                                      [(gpsp, hpsp)])
            # attention PSUM freed; deferred Wo/norm2 for slice 1 with
            # its own banks (no WAR against the gh(0) overlap pools)
            woep = pp_ctx.enter_context(
                tc.tile_pool(name="woe", bufs=2, space="PSUM"))
            emit_wo_norm(1, woep, "wo", woep, "wo")

            # ---- P3: gh(1) interleaved with out-proj(0), then out-proj(1) ----
            with ExitStack() as fctx:
                fpp = fctx.enter_context(
                    tc.tile_pool(name="fpp", bufs=2, space="PSUM"))
                gpsp2 = fctx.enter_context(
                    tc.tile_pool(name="gps2", bufs=1, space="PSUM"))
                hpsp2 = fctx.enter_context(
                    tc.tile_pool(name="hps2", bufs=1, space="PSUM"))
                gpsp3 = fctx.enter_context(
                    tc.tile_pool(name="gps3", bufs=1, space="PSUM"))
                hpsp3 = fctx.enter_context(
                    tc.tile_pool(name="hps3", bufs=1, space="PSUM"))
                tsp = fctx.enter_context(tc.tile_pool(name="tsb", bufs=3))
                gbp = fctx.enter_context(tc.tile_pool(name="gb1", bufs=4))
                wfp = fctx.enter_context(tc.tile_pool(name="wffn", bufs=2))
                wop2 = fctx.enter_context(tc.tile_pool(name="wob2", bufs=2))
                finp = fctx.enter_context(tc.tile_pool(name="fin", bufs=1))
                ghpb = fctx.enter_context(tc.tile_pool(name="ghqb", bufs=1))
                ghq8b = ghpb.tile([P, NHT, 512], FP8, tag="gh8b")
                ghqr8b = ghpb.tile([P, NHT, 512], FP8, tag="ghr8b")
                ghq_sets.append((ghq8b, ghqr8b))
                ybufs = [finp.tile([P, NDT, 512], F32, tag="yb0", name="yb0"),
                         finp.tile([P, NDT, 512], F32, tag="yb1", name="yb1")]
                gh1_pools = [(gpsp, hpsp), (gpsp2, hpsp2), (gpsp3, hpsp3)]
                for k in range(NDT):
                    ffn_gh_jj(1, 2 * k, ghq8b, ghqr8b, wfp, tsp, gbp,
                              gh1_pools)
                    ffn_gh_jj(1, 2 * k + 1, ghq8b, ghqr8b, wfp, tsp, gbp,
                              gh1_pools)
                    ffn_out_fo(0, k, wop2, fpp, ybufs[0])
                nc.gpsimd.dma_start(out=yTr[:, :, 0:512], in_=ybufs[0])
                for fo in range(NDT):
                    ffn_out_fo(1, fo, wop2, fpp, ybufs[1])
                nc.gpsimd.dma_start(out=yTr[:, :, 512:1024], in_=ybufs[1])
